# revision 17
# baseline (speedup 1.0000x reference)
"""Multi-head attention (B=4, S=2048, D=1024, H=16) on 8 TRN2 NeuronCores.

Sharding: core c handles batch b=c//2 and head-group g=c%2 (8 heads, 512 of
the 1024 model dims).  Wq/Wk/Wv column-parallel, Wo row-parallel; the two
head-group partial outputs per batch are summed on the host (no collectives).

Per-core dataflow (all matmuls bf16 in, fp32 PSUM accumulate):
  phase 1: Q.T = (Wq/8) @ x.T   [512,2048]
           K.T = Wk @ x.T       [512,2048]
           V   = x @ Wv.T       [2048,512]   stored head-major with a ones
                                             column and zero padding to 128:
                                             [128, 4, 8, 128] per quarter
  phase 2 (per 512-wide q-block, per HEAD PAIR (2j, 2j+1) sharing m-tile j):
           scoresT[k,q] for both heads of the pair land in one [128,1024]
           PSUM tile (even head at 0, odd at 512) via two K=64 matmuls on
           disjoint PE row groups (tile_position 0 / 64 -> concurrent);
           scores of two consecutive k-tiles are batched so the row groups
           alternate and the weight loads pipeline
           exp(ACT) over the packed tile, then one strided DVE multiply per
           diagonal chunk applies the 0/1 mask to both heads
           raw_h[128,512] += [V_h|1|0pad].T @ expT  (row 64 = softmax denom;
           FD=128 stationary keeps the weight load pipelined)
           attnT = raw[0:64] * (1/raw[64])  (fast reciprocal on the pair,
           gpsimd bcast, DVE mul)
  phase 3 (per q-block, deferred to fill the attention tail on PE):
           outT += Wo_g.T.T @ attnT_cat -> [1024,2048] bf16 partial
Host: out[b] = (partial_g0 + partial_g1).T + bo
"""

import numpy as np
import ml_dtypes
from contextlib import ExitStack

B = 4
S = 2048
D = 1024
H = 16
DK = 64
G = 2                 # head groups
HL = H // G           # heads per core = 8
DL = D // G           # local head dims = 512
QB = 512              # q-block width
CH = 128              # chunk / k-tile width
NKT = S // CH         # 16 k-tiles
NQB = S // QB         # 4 q-blocks
NCORES = 8


def _bf16(x):
    return np.ascontiguousarray(x, dtype=np.float32).astype(ml_dtypes.bfloat16)


def _plan_from_mask(m):
    """m: [S, S] bool, True = masked (scores[q, k] masked).

    Returns (plans, patterns):
      plans[qb][kt] = None (skip) or (c0, nch, [(rel_chunk, uidx), ...])
        c0: first valid 128-chunk index within the q-block, nch: chunk count
      patterns: list of unique [128,128] float32 0/1 valid-masks (scoresT
        orientation: [k_partition, q_free]).
    """
    patterns = []
    pat_index = {}
    plans = []
    for qb in range(NQB):
        row = []
        for kt in range(NKT):
            # scoresT tile: partitions = k in [kt*128, ...), free = q chunk
            sub = m[qb * QB:(qb + 1) * QB, kt * CH:(kt + 1) * CH]  # [q, k]
            valid = (~sub).T  # [k, q] 128 x 512
            nchunks = QB // CH
            kinds = []
            for c in range(nchunks):
                ch = valid[:, c * CH:(c + 1) * CH]
                if ch.all():
                    kinds.append("full")
                elif not ch.any():
                    kinds.append("empty")
                else:
                    kinds.append("mixed")
            not_empty = [c for c in range(nchunks) if kinds[c] != "empty"]
            if not not_empty:
                row.append(None)
                continue
            c0, c1 = not_empty[0], not_empty[-1]
            mixed = []
            for c in range(c0, c1 + 1):
                if kinds[c] == "full":
                    continue
                pat = valid[:, c * CH:(c + 1) * CH].astype(np.float32)
                key = pat.tobytes()
                if key not in pat_index:
                    pat_index[key] = len(patterns)
                    patterns.append(pat)
                mixed.append((c - c0, pat_index[key]))
            row.append((c0, c1 - c0 + 1, mixed))
        plans.append(row)
    return plans, patterns


def _build(plans, n_patterns, guard_empty_rows, has_bias):
    import concourse.bacc as bacc
    import concourse.tile as tile
    from concourse import mybir

    F32 = mybir.dt.float32
    BF16 = mybir.dt.bfloat16
    AF = mybir.ActivationFunctionType

    nc = bacc.Bacc("TRN2", target_bir_lowering=False, debug=False)

    xq = nc.dram_tensor("xq_t", [D, S], BF16, kind="ExternalInput")
    xk = nc.dram_tensor("xk_t", [D, S], BF16, kind="ExternalInput")
    xv = nc.dram_tensor("xv_t", [D, S], BF16, kind="ExternalInput")
    wq = nc.dram_tensor("wq_t", [D, DL], BF16, kind="ExternalInput")
    wk = nc.dram_tensor("wk_t", [D, DL], BF16, kind="ExternalInput")
    wv = nc.dram_tensor("wv_t", [D, DL], BF16, kind="ExternalInput")
    wo = nc.dram_tensor("wo_t", [DL, D], BF16, kind="ExternalInput")
    bq = nc.dram_tensor("bq8", [1, DL], BF16, kind="ExternalInput")
    bk = nc.dram_tensor("bk", [1, DL], BF16, kind="ExternalInput")
    bv = nc.dram_tensor("bv", [1, DL], BF16, kind="ExternalInput")
    onesr = nc.dram_tensor("ones_row", [1, QB], BF16, kind="ExternalInput")
    onesc = nc.dram_tensor("ones_cols", [CH, NKT * HL], BF16, kind="ExternalInput")
    if n_patterns:
        # 0/1 valid patterns, duplicated x2 so one strided DVE multiply
        # masks both heads of a pair
        maskp = nc.dram_tensor("maskp", [CH, n_patterns * 2 * CH], BF16,
                               kind="ExternalInput")
    outT = nc.dram_tensor("outT", [D, S], BF16, kind="ExternalOutput")

    MT = DL // CH      # 4 dq/dcat tiles
    NQU = S // QB      # 4 s-quarters
    NK = D // CH       # 8 contraction tiles
    NPAIR = HL // 2    # 4 head pairs

    with tile.TileContext(nc) as tc, ExitStack() as ctx:
        persist = ctx.enter_context(tc.tile_pool(name="persist", bufs=1))
        xin = ctx.enter_context(tc.tile_pool(name="xin", bufs=24))
        wt = ctx.enter_context(tc.tile_pool(name="wt", bufs=25))
        expp = ctx.enter_context(tc.tile_pool(name="expp", bufs=4))
        attp = ctx.enter_context(tc.tile_pool(name="attp", bufs=4))
        outp = ctx.enter_context(tc.tile_pool(name="outp", bufs=4))
        recp = ctx.enter_context(tc.tile_pool(name="recp", bufs=4))
        ps_mm = ctx.enter_context(tc.tile_pool(name="ps_mm", bufs=2, space="PSUM"))
        ps_sc = ctx.enter_context(tc.tile_pool(name="ps_sc", bufs=2, space="PSUM"))
        ps_raw = ctx.enter_context(tc.tile_pool(name="ps_raw", bufs=2, space="PSUM"))

        # per-(m, quarter) projection output tiles -> fine-grained deps let
        # attention(qb) start as soon as quarters <= qb are projected
        qt_q = {(m, qu): persist.tile([CH, QB], BF16, name=f"qt_{m}_{qu}")
                for m in range(MT) for qu in range(NQU)}
        kt_q = {(m, qu): persist.tile([CH, QB], BF16, name=f"kt_{m}_{qu}")
                for m in range(MT) for qu in range(NQU)}
        # V stationary, head-major: [k, chunk j, head, V(64)|ones|zero pad]
        v_g = [persist.tile([CH, NQU, HL, CH], BF16, name=f"v_g{qu}")
               for qu in range(NQU)]
        wo_all = persist.tile([CH, MT, D], BF16)
        ones_sb = persist.tile([1, QB], BF16)
        bq_sb = persist.tile([1, DL], BF16)
        bk_sb = persist.tile([1, DL], BF16)
        bv_sb = persist.tile([1, DL], BF16)
        if n_patterns:
            mp_sb = persist.tile([CH, n_patterns, 2, CH], BF16)

        nc.sync.dma_start(ones_sb[:], onesr.ap())
        if has_bias[0]:
            nc.sync.dma_start(bq_sb[:], bq.ap())
        if has_bias[1]:
            nc.sync.dma_start(bk_sb[:], bk.ap())
        if has_bias[2]:
            nc.sync.dma_start(bv_sb[:], bv.ap())

        # PE warm-up while the first input DMAs land
        wu_ps = ps_mm.tile([1, QB], F32, tag="mm")
        for _ in range(12):
            nc.tensor.matmul(wu_ps[:], ones_sb[0:1, 0:1], ones_sb[0:1, :],
                             start=True, stop=True, skip_group_check=True)

        # zero the V stationary pad columns (gpsimd; idle during startup)
        for qu in range(NQU):
            nc.gpsimd.memset(v_g[qu][:, :, :, DK + 1:], 0.0)

        # weight tiles: loaded once, reused across quarters
        def load_w(dram):
            tiles = []
            for kt in range(NK):
                wtile = wt.tile([CH, DL], BF16, tag="w")
                nc.sync.dma_start(wtile[:], dram.ap()[kt * CH:(kt + 1) * CH, :])
                tiles.append(wtile)
            return tiles

        def load_x(dram, qu):
            tiles = []
            for kt in range(NK):
                xt = xin.tile([CH, QB], BF16, tag="x")
                nc.sync.dma_start(
                    xt[:], dram.ap()[kt * CH:(kt + 1) * CH,
                                     qu * QB:(qu + 1) * QB])
                tiles.append(xt)
            return tiles

        def proj_qk_quarter(x_dram, w_tiles, bias_sb, dst_map, qu, use_bias):
            x_tiles = load_x(x_dram, qu)
            for m in range(MT):
                ps = ps_mm.tile([CH, QB], F32, tag="mm")
                for kt in range(NK):
                    nc.tensor.matmul(
                        ps[:], w_tiles[kt][:, m * CH:(m + 1) * CH],
                        x_tiles[kt][:], start=(kt == 0),
                        stop=(not use_bias and kt == NK - 1))
                if use_bias:
                    nc.tensor.matmul(
                        ps[:], bias_sb[0:1, m * CH:(m + 1) * CH],
                        ones_sb[0:1, :], start=False, stop=True)
                # ACT does the Q/K PSUM->SBUF casts; DVE is the busier engine
                nc.scalar.copy(out=dst_map[(m, qu)][:], in_=ps[:])

        def proj_v_quarter(wv_tiles, qu, x_tiles=None):
            if x_tiles is None:
                x_tiles = load_x(xv, qu)
            # ones column for this quarter's V tiles
            nc.sync.dma_start(
                v_g[qu][:, :, :, DK:DK + 1],
                onesc.ap()[:, qu * NQU * HL:(qu + 1) * NQU * HL].rearrange(
                    "p (s h o) -> p s h o", h=HL, o=1),
            )
            for j in range(QB // CH):
                ps = ps_mm.tile([CH, DL], F32, tag="mm")
                for kt in range(NK):
                    nc.tensor.matmul(
                        ps[:], x_tiles[kt][:, j * CH:(j + 1) * CH],
                        wv_tiles[kt][:], start=(kt == 0),
                        stop=(not has_bias[2] and kt == NK - 1))
                if has_bias[2]:
                    nc.tensor.matmul(
                        ps[:], ones_sb[0:1, 0:CH], bv_sb[0:1, :],
                        start=False, stop=True)
                nc.vector.tensor_copy(
                    out=v_g[qu][:, j, :, 0:DK],
                    in_=ps[:].rearrange("p (h c) -> p h c", c=DK),
                )

        def attention_qb(qb):
            att = attp.tile([CH, MT, QB], BF16, tag="att")
            units = [(kt,) + plans[qb][kt] for kt in range(NKT)
                     if plans[qb][kt] is not None]
            units = [(kt, c0 * CH, nch * CH, mixed)
                     for (kt, c0, nch, mixed) in units]
            for j in range(NPAIR):
                raw_e = ps_raw.tile([CH, QB], F32, tag="raw")
                raw_o = ps_raw.tile([CH, QB], F32, tag="raw")
                nunit = 0
                for i0 in range(0, len(units), 2):
                    batch = units[i0:i0 + 2]
                    # scores for both packages first: the K=64 matmuls
                    # alternate PE row groups 0/64, so each weight load hides
                    # under the previous matmul and pairs run concurrently
                    scs = []
                    for (kt, o, w, mixed) in batch:
                        sc = ps_sc.tile([CH, 2 * QB], F32, tag="sc")
                        kt_tile = kt_q[(j, kt // 4)]
                        q_tile = qt_q[(j, qb)]
                        ktc = slice((kt % 4) * CH, (kt % 4 + 1) * CH)
                        for h01, base in ((0, 0), (1, QB)):
                            hp = h01 * DK
                            nc.tensor.matmul(
                                sc[:, base:base + w],
                                kt_tile[hp:hp + DK, ktc],
                                q_tile[hp:hp + DK, o:o + w],
                                start=True, stop=True)
                        scs.append(sc)
                    exs = []
                    for (kt, o, w, mixed), sc in zip(batch, scs):
                        ex = expp.tile([CH, 2 * QB], BF16, tag="exp")
                        if w == QB:
                            nc.scalar.activation(ex[:, 0:2 * QB],
                                                 sc[:, 0:2 * QB], AF.Exp)
                        else:
                            nc.scalar.activation(ex[:, 0:w], sc[:, 0:w],
                                                 AF.Exp)
                            nc.scalar.activation(ex[:, QB:QB + w],
                                                 sc[:, QB:QB + w], AF.Exp)
                        exv = ex[:].rearrange("p (a b) -> p a b", a=2)
                        for (rel, uidx) in mixed:
                            nc.vector.tensor_mul(
                                exv[:, :, rel * CH:(rel + 1) * CH],
                                exv[:, :, rel * CH:(rel + 1) * CH],
                                mp_sb[:, uidx, :, :])
                        exs.append(ex)
                    for (kt, o, w, mixed), ex in zip(batch, exs):
                        for h01, base, raw in ((0, 0, raw_e), (1, QB, raw_o)):
                            nc.tensor.matmul(
                                raw[:, o:o + w],
                                v_g[kt // 4][:, kt % 4, 2 * j + h01, :],
                                ex[:, base:base + w],
                                start=(nunit == 0), stop=False,
                                skip_group_check=True)
                        nunit += 1
                # normalize pair -> attnT
                for h01, raw, asl in ((0, raw_e, slice(0, DK)),
                                      (1, raw_o, slice(DK, CH))):
                    # custom-DVE reciprocal needs an SBUF source: stage the
                    # denominator row out of PSUM first
                    den = recp.tile([1, QB], F32, tag="den")
                    if guard_empty_rows:
                        nc.vector.tensor_scalar_max(den[:], raw[DK:DK + 1, :],
                                                    1e-30)
                    else:
                        nc.vector.tensor_copy(den[:], raw[DK:DK + 1, :])
                    rec = recp.tile([1, QB], F32, tag="rec")
                    nc.vector.reciprocal_approx_fast(out=rec[:], in_=den[:])
                    recb = recp.tile([DK, QB], F32, tag="recb")
                    nc.gpsimd.partition_broadcast(recb[:], rec[:])
                    nc.vector.tensor_mul(att[asl, j, :], raw[0:DK, :], recb[:])
            return att

        def outproj_qb(qb, att):
            for mo in range(D // CH):
                ps = ps_mm.tile([CH, QB], F32, tag="mm")
                for ct in range(MT):
                    nc.tensor.matmul(
                        ps[:], wo_all[:, ct, mo * CH:(mo + 1) * CH],
                        att[:, ct, :], start=(ct == 0), stop=(ct == MT - 1),
                        skip_group_check=True)
                ot = outp.tile([CH, QB], BF16, tag="ot")
                nc.vector.tensor_copy(out=ot[:], in_=ps[:])
                nc.sync.dma_start(
                    outT.ap()[mo * CH:(mo + 1) * CH, qb * QB:(qb + 1) * QB],
                    ot[:])

        # ---- emission order sets scheduler priority: proj/out-proj matmuls
        # fill PE bubbles left by exp(ACT)-paced attention packages; the
        # out-projections are deferred so the attention tail still has PE
        # fill work.
        # interleave the first weight/x DMAs so the first projection matmul
        # can start as soon as one (w, x) tile pair lands
        wv_t = []
        xv0 = []
        for kt in range(NK):
            wtile = wt.tile([CH, DL], BF16, tag="w")
            nc.sync.dma_start(wtile[:], wv.ap()[kt * CH:(kt + 1) * CH, :])
            wv_t.append(wtile)
            xt = xin.tile([CH, QB], BF16, tag="x")
            nc.sync.dma_start(xt[:], xv.ap()[kt * CH:(kt + 1) * CH, 0:QB])
            xv0.append(xt)
        proj_v_quarter(wv_t, 0, x_tiles=xv0)
        wk_t = load_w(wk)
        proj_qk_quarter(xk, wk_t, bk_sb, kt_q, 0, has_bias[1])
        wq_t = load_w(wq)
        proj_qk_quarter(xq, wq_t, bq_sb, qt_q, 0, has_bias[0])
        # bulk constants needed from attention onward
        if n_patterns:
            nc.sync.dma_start(mp_sb[:], maskp.ap().rearrange(
                "p (u a f) -> p u a f", a=2, f=CH))
        nc.sync.dma_start(wo_all[:], wo.ap().rearrange("(t p) m -> p t m", p=CH))

        def proj_round(qu):
            proj_v_quarter(wv_t, qu)
            proj_qk_quarter(xk, wk_t, bk_sb, kt_q, qu, has_bias[1])
            proj_qk_quarter(xq, wq_t, bq_sb, qt_q, qu, has_bias[0])

        proj_round(1)
        att0 = attention_qb(0)
        proj_round(2)
        att1 = attention_qb(1)
        # only Q of quarter 3 is needed when attention(3) starts; V/K of
        # quarter 3 feed its last four k-tiles only, so they are emitted
        # after attention(3) as PE fill for its exp-paced stretches
        proj_qk_quarter(xq, wq_t, bq_sb, qt_q, 3, has_bias[0])
        att2 = attention_qb(2)
        outproj_qb(0, att0)
        outproj_qb(1, att1)
        att3 = attention_qb(3)
        proj_v_quarter(wv_t, 3)
        proj_qk_quarter(xk, wk_t, bk_sb, kt_q, 3, has_bias[1])
        outproj_qb(2, att2)
        outproj_qb(3, att3)

    nc.compile()
    return nc


_CACHE = {}
LAST_RESULTS = None


def _install_ntff_shim():
    """Provide antenv.axon_hooks (NTFF profiling) when the image lacks it."""
    import sys, types, ctypes, contextlib
    if "antenv.axon_hooks" in sys.modules:
        return
    import antenv
    mod = types.ModuleType("antenv.axon_hooks")
    state = {"hook": None}
    mod.set_axon_ntff_profile_hook = lambda h: state.__setitem__("hook", h)
    mod.get_axon_ntff_profile_hook = lambda: state["hook"]
    sys.modules["antenv.axon_hooks"] = mod
    antenv.axon_hooks = mod
    try:
        lib = ctypes.CDLL("/opt/axon/libaxon_pjrt.so")
    except OSError:
        return
    if not hasattr(lib, "axon_start_nrt_profile"):
        return
    lib.axon_start_nrt_profile.argtypes = [
        ctypes.POINTER(ctypes.c_int64), ctypes.c_size_t]
    lib.axon_start_nrt_profile.restype = ctypes.c_int64
    lib.axon_stop_nrt_profile.argtypes = [ctypes.c_char_p]
    lib.axon_stop_nrt_profile.restype = ctypes.c_int64

    @contextlib.contextmanager
    def _hook(output_dir, device_ids):
        import jax
        jax.devices()
        if device_ids:
            ids = (ctypes.c_int64 * len(device_ids))(*device_ids)
            rc = lib.axon_start_nrt_profile(ids, len(device_ids))
        else:
            rc = lib.axon_start_nrt_profile(None, 0)
        if rc != 0:
            raise RuntimeError(f"axon_start_nrt_profile rc={rc}")
        try:
            yield
        finally:
            n = lib.axon_stop_nrt_profile(str(output_dir).encode())
            print(f"profile: {n} ntff file(s) in {output_dir}", file=sys.stderr)

    state["hook"] = _hook


def _get_nc(mask2d, has_bias):
    key = (hash(mask2d.tobytes()), has_bias)
    if key not in _CACHE:
        plans, patterns = _plan_from_mask(mask2d)
        # guard against fully-masked rows (reference maps softmax NaN -> 0)
        valid_any = (~mask2d).any(axis=1)
        guard = bool((~valid_any).any())
        _CACHE[key] = (_build(plans, len(patterns), guard, has_bias), patterns)
    return _CACHE[key]


def kernel(query, key, value, mask, Wq, bq, Wk, bk, Wv, bv, Wo, bo):
    from concourse.bass_utils import run_bass_kernel_spmd

    query = np.asarray(query, dtype=np.float32)
    key_ = np.asarray(key, dtype=np.float32)
    value = np.asarray(value, dtype=np.float32)
    mask2d = np.asarray(mask, dtype=bool).reshape(S, S)
    Wq = np.asarray(Wq, dtype=np.float32)
    Wk = np.asarray(Wk, dtype=np.float32)
    Wv = np.asarray(Wv, dtype=np.float32)
    Wo = np.asarray(Wo, dtype=np.float32)
    bq = np.asarray(bq, dtype=np.float32)
    bk = np.asarray(bk, dtype=np.float32)
    bv = np.asarray(bv, dtype=np.float32)
    bo = np.asarray(bo, dtype=np.float32)

    has_bias = (bool(bq.any()), bool(bk.any()), bool(bv.any()))
    nc, patterns = _get_nc(mask2d, has_bias)

    n_pat = len(patterns)
    if n_pat:
        mp = np.empty((CH, n_pat * 2 * CH), np.float32)
        for u, pat in enumerate(patterns):
            mp[:, (2 * u) * CH:(2 * u + 1) * CH] = pat
            mp[:, (2 * u + 1) * CH:(2 * u + 2) * CH] = pat
        mp = mp.astype(ml_dtypes.bfloat16)
    ones_row = np.ones((1, QB), ml_dtypes.bfloat16)
    ones_cols = np.ones((CH, NKT * HL), ml_dtypes.bfloat16)

    in_maps = []
    for c in range(NCORES):
        b, g = divmod(c, 2)
        gsl = slice(DL * g, DL * (g + 1))
        m = {
            "xq_t": _bf16(query[b].T),
            "xk_t": _bf16(key_[b].T),
            "xv_t": _bf16(value[b].T),
            "wq_t": _bf16(Wq[gsl].T * 0.125),
            "wk_t": _bf16(Wk[gsl].T),
            "wv_t": _bf16(Wv[gsl].T),
            "wo_t": _bf16(Wo[:, gsl].T),
            "bq8": _bf16(bq[gsl].reshape(1, DL) * 0.125),
            "bk": _bf16(bk[gsl].reshape(1, DL)),
            "bv": _bf16(bv[gsl].reshape(1, DL)),
            "ones_row": ones_row,
            "ones_cols": ones_cols,
        }
        if n_pat:
            m["maskp"] = mp
        in_maps.append(m)

    import os
    kwargs = {}
    if os.environ.get("BASS_MHA_TRACE"):
        _install_ntff_shim()
        tc_env = os.environ.get("BASS_MHA_TRACE_CORES", "0")
        cores = (list(range(NCORES)) if tc_env == "all"
                 else [int(x) for x in tc_env.split(",")])
        kwargs = dict(trace=True, trace_cores=cores)

    global LAST_RESULTS
    out = np.empty((B, S, D), np.float32)
    for attempt in range(3):
        res = run_bass_kernel_spmd(nc, in_maps, core_ids=list(range(NCORES)),
                                   **kwargs)
        LAST_RESULTS = res
        for b in range(B):
            acc = (res.results[2 * b]["outT"].astype(np.float32)
                   + res.results[2 * b + 1]["outT"].astype(np.float32))
            out[b] = acc.T
        # softmax-bounded outputs are O(10); a bad first execution after NEFF
        # load shows up as huge/NaN values -> rerun
        if np.isfinite(out).all() and np.abs(out).max() < 1e4:
            break
    return out + bo[None, None, :]


# revision 24
# speedup vs baseline: 1.0050x; 1.0050x over previous
"""Multi-head attention (B=4, S=2048, D=1024, H=16) on 8 TRN2 NeuronCores.

Sharding: core c handles batch b=c//2 and head-group g=c%2 (8 heads, 512 of
the 1024 model dims).  Wq/Wk/Wv column-parallel, Wo row-parallel; the two
head-group partial outputs per batch are summed on the host (no collectives).

Per-core dataflow (all matmuls bf16 in, fp32 PSUM accumulate):
  phase 1: Q.T = (Wq/8) @ x.T   [512,2048]
           K.T = Wk @ x.T       [512,2048]
           V   = x @ Wv.T       [2048,512]   stored head-major with a ones
                                             column and zero padding to 128:
                                             [128, 4, 8, 128] per quarter
  phase 2 (per 512-wide q-block, per HEAD PAIR (2j, 2j+1) sharing m-tile j):
           scoresT[k,q] for both heads of the pair land in one [128,1024]
           PSUM tile (even head at 0, odd at 512) via two K=64 matmuls on
           disjoint PE row groups (tile_position 0 / 64 -> concurrent);
           scores of two consecutive k-tiles are batched so the row groups
           alternate and the weight loads pipeline
           exp(ACT) over the packed tile, then one strided DVE multiply per
           diagonal chunk applies the 0/1 mask to both heads
           raw_h[128,512] += [V_h|1|0pad].T @ expT  (row 64 = softmax denom;
           FD=128 stationary keeps the weight load pipelined)
           attnT = raw[0:64] * (1/raw[64])  (fast reciprocal on the pair,
           gpsimd bcast, DVE mul)
  phase 3 (per q-block, deferred to fill the attention tail on PE):
           outT += Wo_g.T.T @ attnT_cat -> [1024,2048] bf16 partial
Host: out[b] = (partial_g0 + partial_g1).T + bo
"""

import numpy as np
import ml_dtypes
from collections import deque
from contextlib import ExitStack

B = 4
S = 2048
D = 1024
H = 16
DK = 64
G = 2                 # head groups
HL = H // G           # heads per core = 8
DL = D // G           # local head dims = 512
QB = 512              # q-block width
CH = 128              # chunk / k-tile width
NKT = S // CH         # 16 k-tiles
NQB = S // QB         # 4 q-blocks
NCORES = 8


def _bf16(x):
    return np.ascontiguousarray(x, dtype=np.float32).astype(ml_dtypes.bfloat16)


def _plan_from_mask(m):
    """m: [S, S] bool, True = masked (scores[q, k] masked).

    Returns (plans, patterns):
      plans[qb][kt] = None (skip) or (c0, nch, [(rel_chunk, uidx), ...])
        c0: first valid 128-chunk index within the q-block, nch: chunk count
      patterns: list of unique [128,128] float32 0/1 valid-masks (scoresT
        orientation: [k_partition, q_free]).
    """
    patterns = []
    pat_index = {}
    plans = []
    for qb in range(NQB):
        row = []
        for kt in range(NKT):
            # scoresT tile: partitions = k in [kt*128, ...), free = q chunk
            sub = m[qb * QB:(qb + 1) * QB, kt * CH:(kt + 1) * CH]  # [q, k]
            valid = (~sub).T  # [k, q] 128 x 512
            nchunks = QB // CH
            kinds = []
            for c in range(nchunks):
                ch = valid[:, c * CH:(c + 1) * CH]
                if ch.all():
                    kinds.append("full")
                elif not ch.any():
                    kinds.append("empty")
                else:
                    kinds.append("mixed")
            not_empty = [c for c in range(nchunks) if kinds[c] != "empty"]
            if not not_empty:
                row.append(None)
                continue
            c0, c1 = not_empty[0], not_empty[-1]
            mixed = []
            for c in range(c0, c1 + 1):
                if kinds[c] == "full":
                    continue
                pat = valid[:, c * CH:(c + 1) * CH].astype(np.float32)
                key = pat.tobytes()
                if key not in pat_index:
                    pat_index[key] = len(patterns)
                    patterns.append(pat)
                mixed.append((c - c0, pat_index[key]))
            row.append((c0, c1 - c0 + 1, mixed))
        plans.append(row)
    return plans, patterns


def _build(plans, n_patterns, guard_empty_rows, has_bias):
    import concourse.bacc as bacc
    import concourse.tile as tile
    from concourse import mybir

    F32 = mybir.dt.float32
    BF16 = mybir.dt.bfloat16
    AF = mybir.ActivationFunctionType

    nc = bacc.Bacc("TRN2", target_bir_lowering=False, debug=False)

    xq = nc.dram_tensor("xq_t", [D, S], BF16, kind="ExternalInput")
    xk = nc.dram_tensor("xk_t", [D, S], BF16, kind="ExternalInput")
    xv = nc.dram_tensor("xv_t", [D, S], BF16, kind="ExternalInput")
    wq = nc.dram_tensor("wq_t", [D, DL], BF16, kind="ExternalInput")
    wk = nc.dram_tensor("wk_t", [D, DL], BF16, kind="ExternalInput")
    wv = nc.dram_tensor("wv_t", [D, DL], BF16, kind="ExternalInput")
    wo = nc.dram_tensor("wo_t", [DL, D], BF16, kind="ExternalInput")
    bq = nc.dram_tensor("bq8", [1, DL], BF16, kind="ExternalInput")
    bk = nc.dram_tensor("bk", [1, DL], BF16, kind="ExternalInput")
    bv = nc.dram_tensor("bv", [1, DL], BF16, kind="ExternalInput")
    onesr = nc.dram_tensor("ones_row", [1, QB], BF16, kind="ExternalInput")
    onesc = nc.dram_tensor("ones_cols", [CH, NKT * HL], BF16, kind="ExternalInput")
    if n_patterns:
        # 0/1 valid patterns, duplicated x2 so one strided DVE multiply
        # masks both heads of a pair
        maskp = nc.dram_tensor("maskp", [CH, n_patterns * 2 * CH], BF16,
                               kind="ExternalInput")
    outT = nc.dram_tensor("outT", [D, S], BF16, kind="ExternalOutput")

    MT = DL // CH      # 4 dq/dcat tiles
    NQU = S // QB      # 4 s-quarters
    NK = D // CH       # 8 contraction tiles
    NPAIR = HL // 2    # 4 head pairs

    with tile.TileContext(nc) as tc, ExitStack() as ctx:
        persist = ctx.enter_context(tc.tile_pool(name="persist", bufs=1))
        xin = ctx.enter_context(tc.tile_pool(name="xin", bufs=24))
        wt = ctx.enter_context(tc.tile_pool(name="wt", bufs=25))
        expp = ctx.enter_context(tc.tile_pool(name="expp", bufs=4))
        attp = ctx.enter_context(tc.tile_pool(name="attp", bufs=4))
        outp = ctx.enter_context(tc.tile_pool(name="outp", bufs=4))
        recp = ctx.enter_context(tc.tile_pool(name="recp", bufs=4))
        ps_mm = ctx.enter_context(tc.tile_pool(name="ps_mm", bufs=2, space="PSUM"))
        ps_sc = ctx.enter_context(tc.tile_pool(name="ps_sc", bufs=2, space="PSUM"))
        ps_raw = ctx.enter_context(tc.tile_pool(name="ps_raw", bufs=2, space="PSUM"))

        # per-(m, quarter) projection output tiles -> fine-grained deps let
        # attention(qb) start as soon as quarters <= qb are projected
        qt_q = {(m, qu): persist.tile([CH, QB], BF16, name=f"qt_{m}_{qu}")
                for m in range(MT) for qu in range(NQU)}
        kt_q = {(m, qu): persist.tile([CH, QB], BF16, name=f"kt_{m}_{qu}")
                for m in range(MT) for qu in range(NQU)}
        # V stationary, head-major: [k, chunk j, head, V(64)|ones|zero pad]
        v_g = [persist.tile([CH, NQU, HL, CH], BF16, name=f"v_g{qu}")
               for qu in range(NQU)]
        wo_all = persist.tile([CH, MT, D], BF16)
        ones_sb = persist.tile([1, QB], BF16)
        bq_sb = persist.tile([1, DL], BF16)
        bk_sb = persist.tile([1, DL], BF16)
        bv_sb = persist.tile([1, DL], BF16)
        if n_patterns:
            mp_sb = persist.tile([CH, n_patterns, 2, CH], BF16)

        nc.sync.dma_start(ones_sb[:], onesr.ap())
        if has_bias[0]:
            nc.sync.dma_start(bq_sb[:], bq.ap())
        if has_bias[1]:
            nc.sync.dma_start(bk_sb[:], bk.ap())
        if has_bias[2]:
            nc.sync.dma_start(bv_sb[:], bv.ap())

        # PE warm-up while the first input DMAs land
        wu_ps = ps_mm.tile([1, QB], F32, tag="mm")
        for _ in range(12):
            nc.tensor.matmul(wu_ps[:], ones_sb[0:1, 0:1], ones_sb[0:1, :],
                             start=True, stop=True, skip_group_check=True)

        # zero the V stationary pad columns (gpsimd; idle during startup)
        for qu in range(NQU):
            nc.gpsimd.memset(v_g[qu][:, :, :, DK + 1:], 0.0)

        # weight tiles: loaded once, reused across quarters
        def load_w(dram):
            tiles = []
            for kt in range(NK):
                wtile = wt.tile([CH, DL], BF16, tag="w")
                nc.sync.dma_start(wtile[:], dram.ap()[kt * CH:(kt + 1) * CH, :])
                tiles.append(wtile)
            return tiles

        def load_x(dram, qu):
            tiles = []
            for kt in range(NK):
                xt = xin.tile([CH, QB], BF16, tag="x")
                nc.sync.dma_start(
                    xt[:], dram.ap()[kt * CH:(kt + 1) * CH,
                                     qu * QB:(qu + 1) * QB])
                tiles.append(xt)
            return tiles

        # ---- fill queue: projection / out-projection matmul bundles that
        # attention interleaves into its exp(ACT)-paced stretches so the PE
        # never idles and HAM stays warm.  Thunks emit instructions when
        # popped; emission order sets scheduler priority.
        fill_q = deque()
        fill_credit = [0.0]

        def fill_step(deficit_ns):
            fill_credit[0] += deficit_ns
            while fill_q and fill_credit[0] >= fill_q[0][0]:
                cost, thunk = fill_q.popleft()
                thunk()
                fill_credit[0] -= cost

        def fill_flush():
            while fill_q:
                fill_q.popleft()[1]()
            fill_credit[0] = 0.0

        def proj_qk_m(w_tiles, bias_sb, dst_map, qu, use_bias, m, x_tiles):
            ps = ps_mm.tile([CH, QB], F32, tag="mm")
            for kt in range(NK):
                nc.tensor.matmul(
                    ps[:], w_tiles[kt][:, m * CH:(m + 1) * CH],
                    x_tiles[kt][:], start=(kt == 0),
                    stop=(not use_bias and kt == NK - 1))
            if use_bias:
                nc.tensor.matmul(
                    ps[:], bias_sb[0:1, m * CH:(m + 1) * CH],
                    ones_sb[0:1, :], start=False, stop=True)
            nc.vector.tensor_copy(out=dst_map[(m, qu)][:], in_=ps[:])

        def proj_qk_quarter(x_dram, w_tiles, bias_sb, dst_map, qu, use_bias,
                            push=False):
            x_tiles = load_x(x_dram, qu)
            for m in range(MT):
                if push:
                    fill_q.append((1800, lambda m=m: proj_qk_m(
                        w_tiles, bias_sb, dst_map, qu, use_bias, m, x_tiles)))
                else:
                    proj_qk_m(w_tiles, bias_sb, dst_map, qu, use_bias, m,
                              x_tiles)

        def proj_v_j(wv_tiles, qu, j, x_tiles):
            ps = ps_mm.tile([CH, DL], F32, tag="mm")
            for kt in range(NK):
                nc.tensor.matmul(
                    ps[:], x_tiles[kt][:, j * CH:(j + 1) * CH],
                    wv_tiles[kt][:], start=(kt == 0),
                    stop=(not has_bias[2] and kt == NK - 1))
            if has_bias[2]:
                nc.tensor.matmul(
                    ps[:], ones_sb[0:1, 0:CH], bv_sb[0:1, :],
                    start=False, stop=True)
            nc.vector.tensor_copy(
                out=v_g[qu][:, j, :, 0:DK],
                in_=ps[:].rearrange("p (h c) -> p h c", c=DK),
            )

        def proj_v_quarter(wv_tiles, qu, x_tiles=None, push=False):
            if x_tiles is None:
                x_tiles = load_x(xv, qu)
            # ones column for this quarter's V tiles
            nc.sync.dma_start(
                v_g[qu][:, :, :, DK:DK + 1],
                onesc.ap()[:, qu * NQU * HL:(qu + 1) * NQU * HL].rearrange(
                    "p (s h o) -> p s h o", h=HL, o=1),
            )
            for j in range(QB // CH):
                if push:
                    fill_q.append((1800, lambda j=j: proj_v_j(
                        wv_tiles, qu, j, x_tiles)))
                else:
                    proj_v_j(wv_tiles, qu, j, x_tiles)

        def attention_qb(qb):
            att = attp.tile([CH, MT, QB], BF16, tag="att")
            units = [(kt,) + plans[qb][kt] for kt in range(NKT)
                     if plans[qb][kt] is not None]
            units = [(kt, c0 * CH, nch * CH, mixed)
                     for (kt, c0, nch, mixed) in units]
            for j in range(NPAIR):
                raw_e = ps_raw.tile([CH, QB], F32, tag="raw")
                raw_o = ps_raw.tile([CH, QB], F32, tag="raw")
                nunit = 0
                for i0 in range(0, len(units), 2):
                    batch = units[i0:i0 + 2]
                    # scores for both packages first: the K=64 matmuls
                    # alternate PE row groups 0/64, so each weight load hides
                    # under the previous matmul and pairs run concurrently.
                    # the odd head packs at offset w when both windows fit in
                    # one PSUM bank (one exp call), else at the bank boundary
                    deficit = 0.0
                    scs = []
                    for (kt, o, w, mixed) in batch:
                        # the odd head always lands in its own PSUM bank:
                        # the two score matmuls run concurrently on disjoint
                        # row groups, and concurrent drains into ONE bank
                        # hard-fault the PSUM (observed as a device hang)
                        odd_off = QB
                        sc = ps_sc.tile([CH, 2 * QB], F32, tag="sc")
                        kt_tile = kt_q[(j, kt // 4)]
                        q_tile = qt_q[(j, qb)]
                        ktc = slice((kt % 4) * CH, (kt % 4 + 1) * CH)
                        for h01, base in ((0, 0), (1, odd_off)):
                            hp = h01 * DK
                            nc.tensor.matmul(
                                sc[:, base:base + w],
                                kt_tile[hp:hp + DK, ktc],
                                q_tile[hp:hp + DK, o:o + w],
                                start=True, stop=True)
                        scs.append((sc, odd_off))
                        ncalls = 1 if odd_off == w else 2
                        deficit += ((2 * w + 313 * ncalls) / 1.2
                                    - (1.2 * w / 2.4 + 2 * (w / 2.4 + 15)))
                    exs = []
                    for (kt, o, w, mixed), (sc, odd_off) in zip(batch, scs):
                        ex = expp.tile([CH, 2 * QB], BF16, tag="exp")
                        if odd_off == w:
                            nc.scalar.activation(ex[:, 0:2 * w], sc[:, 0:2 * w],
                                                 AF.Exp)
                        else:
                            nc.scalar.activation(ex[:, 0:w], sc[:, 0:w],
                                                 AF.Exp)
                            nc.scalar.activation(ex[:, QB:QB + w],
                                                 sc[:, QB:QB + w], AF.Exp)
                        if mixed:
                            exv = ex[:, 0:2 * odd_off].rearrange(
                                "p (a b) -> p a b", a=2)
                            for (rel, uidx) in mixed:
                                nc.vector.tensor_mul(
                                    exv[:, :, rel * CH:(rel + 1) * CH],
                                    exv[:, :, rel * CH:(rel + 1) * CH],
                                    mp_sb[:, uidx, :, :])
                        exs.append((ex, odd_off))
                    for (kt, o, w, mixed), (ex, odd_off) in zip(batch, exs):
                        for h01, base, raw in ((0, 0, raw_e),
                                               (1, odd_off, raw_o)):
                            nc.tensor.matmul(
                                raw[:, o:o + w],
                                v_g[kt // 4][:, kt % 4, 2 * j + h01, :],
                                ex[:, base:base + w],
                                start=(nunit == 0), stop=False,
                                skip_group_check=True)
                        nunit += 1
                    fill_step(deficit)
                # normalize pair -> attnT
                for h01, raw, asl in ((0, raw_e, slice(0, DK)),
                                      (1, raw_o, slice(DK, CH))):
                    # custom-DVE reciprocal needs an SBUF source: stage the
                    # denominator row out of PSUM first
                    den = recp.tile([1, QB], F32, tag="den")
                    if guard_empty_rows:
                        nc.vector.tensor_scalar_max(den[:], raw[DK:DK + 1, :],
                                                    1e-30)
                    else:
                        nc.vector.tensor_copy(den[:], raw[DK:DK + 1, :])
                    rec = recp.tile([1, QB], F32, tag="rec")
                    nc.vector.reciprocal_approx_fast(out=rec[:], in_=den[:])
                    recb = recp.tile([DK, QB], F32, tag="recb")
                    nc.gpsimd.partition_broadcast(recb[:], rec[:])
                    nc.vector.tensor_mul(att[asl, j, :], raw[0:DK, :], recb[:])
            return att

        def outproj_mo(qb, att, mo):
            ps = ps_mm.tile([CH, QB], F32, tag="mm")
            for ct in range(MT):
                nc.tensor.matmul(
                    ps[:], wo_all[:, ct, mo * CH:(mo + 1) * CH],
                    att[:, ct, :], start=(ct == 0), stop=(ct == MT - 1),
                    skip_group_check=True)
            ot = outp.tile([CH, QB], BF16, tag="ot")
            nc.vector.tensor_copy(out=ot[:], in_=ps[:])
            nc.sync.dma_start(
                outT.ap()[mo * CH:(mo + 1) * CH, qb * QB:(qb + 1) * QB],
                ot[:])

        def outproj_qb(qb, att, push=False):
            for mo in range(D // CH):
                if push:
                    fill_q.append((1000, lambda mo=mo: outproj_mo(qb, att, mo)))
                else:
                    outproj_mo(qb, att, mo)

        # ---- emission order sets scheduler priority: proj/out-proj matmuls
        # fill PE bubbles left by exp(ACT)-paced attention packages; the
        # out-projections are deferred so the attention tail still has PE
        # fill work.
        # interleave the first weight/x DMAs so the first projection matmul
        # can start as soon as one (w, x) tile pair lands
        wv_t = []
        xv0 = []
        for kt in range(NK):
            wtile = wt.tile([CH, DL], BF16, tag="w")
            nc.sync.dma_start(wtile[:], wv.ap()[kt * CH:(kt + 1) * CH, :])
            wv_t.append(wtile)
            xt = xin.tile([CH, QB], BF16, tag="x")
            nc.sync.dma_start(xt[:], xv.ap()[kt * CH:(kt + 1) * CH, 0:QB])
            xv0.append(xt)
        proj_v_quarter(wv_t, 0, x_tiles=xv0)
        wk_t = load_w(wk)
        proj_qk_quarter(xk, wk_t, bk_sb, kt_q, 0, has_bias[1])
        wq_t = load_w(wq)
        proj_qk_quarter(xq, wq_t, bq_sb, qt_q, 0, has_bias[0])
        # bulk constants needed from attention onward
        if n_patterns:
            nc.sync.dma_start(mp_sb[:], maskp.ap().rearrange(
                "p (u a f) -> p u a f", a=2, f=CH))
        nc.sync.dma_start(wo_all[:], wo.ap().rearrange("(t p) m -> p t m", p=CH))

        # direct emission; scheduler fills attention's exp-paced PE bubbles
        # with the most recently emitted proj/out-proj work.  V/K of quarter
        # 3 feed only attention(3)'s last four k-tiles, so they are emitted
        # after it as tail fill.
        proj_v_quarter(wv_t, 1)
        proj_qk_quarter(xk, wk_t, bk_sb, kt_q, 1, has_bias[1])
        proj_qk_quarter(xq, wq_t, bq_sb, qt_q, 1, has_bias[0])
        att0 = attention_qb(0)
        proj_v_quarter(wv_t, 2)
        proj_qk_quarter(xk, wk_t, bk_sb, kt_q, 2, has_bias[1])
        proj_qk_quarter(xq, wq_t, bq_sb, qt_q, 2, has_bias[0])
        att1 = attention_qb(1)
        proj_qk_quarter(xq, wq_t, bq_sb, qt_q, 3, has_bias[0])
        att2 = attention_qb(2)
        outproj_qb(0, att0)
        outproj_qb(1, att1)
        att3 = attention_qb(3)
        proj_v_quarter(wv_t, 3)
        proj_qk_quarter(xk, wk_t, bk_sb, kt_q, 3, has_bias[1])
        outproj_qb(2, att2)
        outproj_qb(3, att3)

    nc.compile()
    return nc


_CACHE = {}
LAST_RESULTS = None


def _install_ntff_shim():
    """Provide antenv.axon_hooks (NTFF profiling) when the image lacks it."""
    import sys, types, ctypes, contextlib
    if "antenv.axon_hooks" in sys.modules:
        return
    import antenv
    mod = types.ModuleType("antenv.axon_hooks")
    state = {"hook": None}
    mod.set_axon_ntff_profile_hook = lambda h: state.__setitem__("hook", h)
    mod.get_axon_ntff_profile_hook = lambda: state["hook"]
    sys.modules["antenv.axon_hooks"] = mod
    antenv.axon_hooks = mod
    try:
        lib = ctypes.CDLL("/opt/axon/libaxon_pjrt.so")
    except OSError:
        return
    if not hasattr(lib, "axon_start_nrt_profile"):
        return
    lib.axon_start_nrt_profile.argtypes = [
        ctypes.POINTER(ctypes.c_int64), ctypes.c_size_t]
    lib.axon_start_nrt_profile.restype = ctypes.c_int64
    lib.axon_stop_nrt_profile.argtypes = [ctypes.c_char_p]
    lib.axon_stop_nrt_profile.restype = ctypes.c_int64

    @contextlib.contextmanager
    def _hook(output_dir, device_ids):
        import jax
        jax.devices()
        if device_ids:
            ids = (ctypes.c_int64 * len(device_ids))(*device_ids)
            rc = lib.axon_start_nrt_profile(ids, len(device_ids))
        else:
            rc = lib.axon_start_nrt_profile(None, 0)
        if rc != 0:
            raise RuntimeError(f"axon_start_nrt_profile rc={rc}")
        try:
            yield
        finally:
            n = lib.axon_stop_nrt_profile(str(output_dir).encode())
            print(f"profile: {n} ntff file(s) in {output_dir}", file=sys.stderr)

    state["hook"] = _hook


def _get_nc(mask2d, has_bias):
    key = (hash(mask2d.tobytes()), has_bias)
    if key not in _CACHE:
        plans, patterns = _plan_from_mask(mask2d)
        # guard against fully-masked rows (reference maps softmax NaN -> 0)
        valid_any = (~mask2d).any(axis=1)
        guard = bool((~valid_any).any())
        _CACHE[key] = (_build(plans, len(patterns), guard, has_bias), patterns)
    return _CACHE[key]


def kernel(query, key, value, mask, Wq, bq, Wk, bk, Wv, bv, Wo, bo):
    from concourse.bass_utils import run_bass_kernel_spmd

    query = np.asarray(query, dtype=np.float32)
    key_ = np.asarray(key, dtype=np.float32)
    value = np.asarray(value, dtype=np.float32)
    mask2d = np.asarray(mask, dtype=bool).reshape(S, S)
    Wq = np.asarray(Wq, dtype=np.float32)
    Wk = np.asarray(Wk, dtype=np.float32)
    Wv = np.asarray(Wv, dtype=np.float32)
    Wo = np.asarray(Wo, dtype=np.float32)
    bq = np.asarray(bq, dtype=np.float32)
    bk = np.asarray(bk, dtype=np.float32)
    bv = np.asarray(bv, dtype=np.float32)
    bo = np.asarray(bo, dtype=np.float32)

    has_bias = (bool(bq.any()), bool(bk.any()), bool(bv.any()))
    nc, patterns = _get_nc(mask2d, has_bias)

    n_pat = len(patterns)
    if n_pat:
        mp = np.empty((CH, n_pat * 2 * CH), np.float32)
        for u, pat in enumerate(patterns):
            mp[:, (2 * u) * CH:(2 * u + 1) * CH] = pat
            mp[:, (2 * u + 1) * CH:(2 * u + 2) * CH] = pat
        mp = mp.astype(ml_dtypes.bfloat16)
    ones_row = np.ones((1, QB), ml_dtypes.bfloat16)
    ones_cols = np.ones((CH, NKT * HL), ml_dtypes.bfloat16)

    in_maps = []
    for c in range(NCORES):
        b, g = divmod(c, 2)
        gsl = slice(DL * g, DL * (g + 1))
        m = {
            "xq_t": _bf16(query[b].T),
            "xk_t": _bf16(key_[b].T),
            "xv_t": _bf16(value[b].T),
            "wq_t": _bf16(Wq[gsl].T * 0.125),
            "wk_t": _bf16(Wk[gsl].T),
            "wv_t": _bf16(Wv[gsl].T),
            "wo_t": _bf16(Wo[:, gsl].T),
            "bq8": _bf16(bq[gsl].reshape(1, DL) * 0.125),
            "bk": _bf16(bk[gsl].reshape(1, DL)),
            "bv": _bf16(bv[gsl].reshape(1, DL)),
            "ones_row": ones_row,
            "ones_cols": ones_cols,
        }
        if n_pat:
            m["maskp"] = mp
        in_maps.append(m)

    import os
    kwargs = {}
    if os.environ.get("BASS_MHA_TRACE"):
        _install_ntff_shim()
        tc_env = os.environ.get("BASS_MHA_TRACE_CORES", "0")
        cores = (list(range(NCORES)) if tc_env == "all"
                 else [int(x) for x in tc_env.split(",")])
        kwargs = dict(trace=True, trace_cores=cores)

    global LAST_RESULTS
    out = np.empty((B, S, D), np.float32)
    for attempt in range(3):
        res = run_bass_kernel_spmd(nc, in_maps, core_ids=list(range(NCORES)),
                                   **kwargs)
        LAST_RESULTS = res
        for b in range(B):
            acc = (res.results[2 * b]["outT"].astype(np.float32)
                   + res.results[2 * b + 1]["outT"].astype(np.float32))
            out[b] = acc.T
        # softmax-bounded outputs are O(10); a bad first execution after NEFF
        # load shows up as huge/NaN values -> rerun
        if np.isfinite(out).all() and np.abs(out).max() < 1e4:
            break
    return out + bo[None, None, :]


# revision 25
# speedup vs baseline: 1.0304x; 1.0253x over previous
"""Multi-head attention (B=4, S=2048, D=1024, H=16) on 8 TRN2 NeuronCores.

Sharding: core c handles batch b=c//2 and head-group g=c%2 (8 heads, 512 of
the 1024 model dims).  Wq/Wk/Wv column-parallel, Wo row-parallel; the two
head-group partial outputs per batch are summed on the host (no collectives).

Per-core dataflow (all matmuls bf16 in, fp32 PSUM accumulate):
  phase 1: Q.T = (Wq/8) @ x.T   [512,2048]
           K.T = Wk @ x.T       [512,2048]
           V   = x @ Wv.T       [2048,512]   stored head-major with a ones
                                             column and zero padding to 128:
                                             [128, 4, 8, 128] per quarter
  phase 2 (per 512-wide q-block, per HEAD PAIR (2j, 2j+1) sharing m-tile j):
           scoresT[k,q] for both heads of the pair land in one [128,1024]
           PSUM tile (even head at 0, odd at 512) via two K=64 matmuls on
           disjoint PE row groups (tile_position 0 / 64 -> concurrent);
           scores of two consecutive k-tiles are batched so the row groups
           alternate and the weight loads pipeline
           exp(ACT) over the packed tile, then one strided DVE multiply per
           diagonal chunk applies the 0/1 mask to both heads
           raw_h[128,512] += [V_h|1|0pad].T @ expT  (row 64 = softmax denom;
           FD=128 stationary keeps the weight load pipelined)
           attnT = raw[0:64] * (1/raw[64])  (fast reciprocal on the pair,
           gpsimd bcast, DVE mul)
  phase 3 (per q-block, deferred to fill the attention tail on PE):
           outT += Wo_g.T.T @ attnT_cat -> [1024,2048] bf16 partial
Host: out[b] = (partial_g0 + partial_g1).T + bo
"""

import numpy as np
import ml_dtypes
from collections import deque
from contextlib import ExitStack

B = 4
S = 2048
D = 1024
H = 16
DK = 64
G = 2                 # head groups
HL = H // G           # heads per core = 8
DL = D // G           # local head dims = 512
QB = 512              # q-block width
CH = 128              # chunk / k-tile width
NKT = S // CH         # 16 k-tiles
NQB = S // QB         # 4 q-blocks
NCORES = 8


def _bf16(x):
    return np.ascontiguousarray(x, dtype=np.float32).astype(ml_dtypes.bfloat16)


def _plan_from_mask(m):
    """m: [S, S] bool, True = masked (scores[q, k] masked).

    Returns (plans, patterns):
      plans[qb][kt] = None (skip) or (c0, nch, [(rel_chunk, uidx), ...])
        c0: first valid 128-chunk index within the q-block, nch: chunk count
      patterns: list of unique [128,128] float32 0/1 valid-masks (scoresT
        orientation: [k_partition, q_free]).
    """
    patterns = []
    pat_index = {}
    plans = []
    for qb in range(NQB):
        row = []
        for kt in range(NKT):
            # scoresT tile: partitions = k in [kt*128, ...), free = q chunk
            sub = m[qb * QB:(qb + 1) * QB, kt * CH:(kt + 1) * CH]  # [q, k]
            valid = (~sub).T  # [k, q] 128 x 512
            nchunks = QB // CH
            kinds = []
            for c in range(nchunks):
                ch = valid[:, c * CH:(c + 1) * CH]
                if ch.all():
                    kinds.append("full")
                elif not ch.any():
                    kinds.append("empty")
                else:
                    kinds.append("mixed")
            not_empty = [c for c in range(nchunks) if kinds[c] != "empty"]
            if not not_empty:
                row.append(None)
                continue
            c0, c1 = not_empty[0], not_empty[-1]
            mixed = []
            for c in range(c0, c1 + 1):
                if kinds[c] == "full":
                    continue
                pat = valid[:, c * CH:(c + 1) * CH].astype(np.float32)
                key = pat.tobytes()
                if key not in pat_index:
                    pat_index[key] = len(patterns)
                    patterns.append(pat)
                mixed.append((c - c0, pat_index[key]))
            row.append((c0, c1 - c0 + 1, mixed))
        plans.append(row)
    return plans, patterns


def _build(plans, n_patterns, guard_empty_rows, has_bias):
    import concourse.bacc as bacc
    import concourse.tile as tile
    from concourse import mybir

    F32 = mybir.dt.float32
    BF16 = mybir.dt.bfloat16
    AF = mybir.ActivationFunctionType

    nc = bacc.Bacc("TRN2", target_bir_lowering=False, debug=False)

    xq = nc.dram_tensor("xq_t", [D, S], BF16, kind="ExternalInput")
    xk = nc.dram_tensor("xk_t", [D, S], BF16, kind="ExternalInput")
    xv = nc.dram_tensor("xv_t", [D, S], BF16, kind="ExternalInput")
    wq = nc.dram_tensor("wq_t", [D, DL], BF16, kind="ExternalInput")
    wk = nc.dram_tensor("wk_t", [D, DL], BF16, kind="ExternalInput")
    wv = nc.dram_tensor("wv_t", [D, DL], BF16, kind="ExternalInput")
    wo = nc.dram_tensor("wo_t", [DL, D], BF16, kind="ExternalInput")
    bq = nc.dram_tensor("bq8", [1, DL], BF16, kind="ExternalInput")
    bk = nc.dram_tensor("bk", [1, DL], BF16, kind="ExternalInput")
    bv = nc.dram_tensor("bv", [1, DL], BF16, kind="ExternalInput")
    onesr = nc.dram_tensor("ones_row", [1, QB], BF16, kind="ExternalInput")
    onesc = nc.dram_tensor("ones_cols", [CH, NKT * HL], BF16, kind="ExternalInput")
    if n_patterns:
        # 0/1 valid patterns, duplicated x2 so one strided DVE multiply
        # masks both heads of a pair
        maskp = nc.dram_tensor("maskp", [CH, n_patterns * 2 * CH], BF16,
                               kind="ExternalInput")
    outT = nc.dram_tensor("outT", [D, S], BF16, kind="ExternalOutput")

    MT = DL // CH      # 4 dq/dcat tiles
    NQU = S // QB      # 4 s-quarters
    NK = D // CH       # 8 contraction tiles
    NPAIR = HL // 2    # 4 head pairs

    with tile.TileContext(nc) as tc, ExitStack() as ctx:
        persist = ctx.enter_context(tc.tile_pool(name="persist", bufs=1))
        xin = ctx.enter_context(tc.tile_pool(name="xin", bufs=24))
        wt = ctx.enter_context(tc.tile_pool(name="wt", bufs=25))
        expp = ctx.enter_context(tc.tile_pool(name="expp", bufs=4))
        attp = ctx.enter_context(tc.tile_pool(name="attp", bufs=4))
        outp = ctx.enter_context(tc.tile_pool(name="outp", bufs=4))
        recp = ctx.enter_context(tc.tile_pool(name="recp", bufs=4))
        ps_mm = ctx.enter_context(tc.tile_pool(name="ps_mm", bufs=2, space="PSUM"))
        ps_sc = ctx.enter_context(tc.tile_pool(name="ps_sc", bufs=2, space="PSUM"))
        ps_raw = ctx.enter_context(tc.tile_pool(name="ps_raw", bufs=2, space="PSUM"))

        # per-(m, quarter) projection output tiles -> fine-grained deps let
        # attention(qb) start as soon as quarters <= qb are projected
        qt_q = {(m, qu): persist.tile([CH, QB], BF16, name=f"qt_{m}_{qu}")
                for m in range(MT) for qu in range(NQU)}
        kt_q = {(m, qu): persist.tile([CH, QB], BF16, name=f"kt_{m}_{qu}")
                for m in range(MT) for qu in range(NQU)}
        # V stationary, head-major: [k, chunk j, head, V(64)|ones|zero pad]
        v_g = [persist.tile([CH, NQU, HL, CH], BF16, name=f"v_g{qu}")
               for qu in range(NQU)]
        wo_all = persist.tile([CH, MT, D], BF16)
        ones_sb = persist.tile([1, QB], BF16)
        bq_sb = persist.tile([1, DL], BF16)
        bk_sb = persist.tile([1, DL], BF16)
        bv_sb = persist.tile([1, DL], BF16)
        if n_patterns:
            mp_sb = persist.tile([CH, n_patterns, 2, CH], BF16)

        nc.sync.dma_start(ones_sb[:], onesr.ap())
        if has_bias[0]:
            nc.sync.dma_start(bq_sb[:], bq.ap())
        if has_bias[1]:
            nc.sync.dma_start(bk_sb[:], bk.ap())
        if has_bias[2]:
            nc.sync.dma_start(bv_sb[:], bv.ap())

        # PE warm-up while the first input DMAs land
        wu_ps = ps_mm.tile([1, QB], F32, tag="mm")
        for _ in range(12):
            nc.tensor.matmul(wu_ps[:], ones_sb[0:1, 0:1], ones_sb[0:1, :],
                             start=True, stop=True, skip_group_check=True)

        # zero the V stationary pad columns (gpsimd; idle during startup)
        for qu in range(NQU):
            nc.gpsimd.memset(v_g[qu][:, :, :, DK + 1:], 0.0)

        # weight tiles: loaded once, reused across quarters
        def load_w(dram):
            tiles = []
            for kt in range(NK):
                wtile = wt.tile([CH, DL], BF16, tag="w")
                nc.sync.dma_start(wtile[:], dram.ap()[kt * CH:(kt + 1) * CH, :])
                tiles.append(wtile)
            return tiles

        def load_x(dram, qu):
            tiles = []
            for kt in range(NK):
                xt = xin.tile([CH, QB], BF16, tag="x")
                nc.sync.dma_start(
                    xt[:], dram.ap()[kt * CH:(kt + 1) * CH,
                                     qu * QB:(qu + 1) * QB])
                tiles.append(xt)
            return tiles

        # ---- fill queue: projection / out-projection matmul bundles that
        # attention interleaves into its exp(ACT)-paced stretches so the PE
        # never idles and HAM stays warm.  Thunks emit instructions when
        # popped; emission order sets scheduler priority.
        fill_q = deque()
        fill_credit = [0.0]

        def fill_step(deficit_ns):
            fill_credit[0] += deficit_ns
            while fill_q and fill_credit[0] >= fill_q[0][0]:
                cost, thunk = fill_q.popleft()
                thunk()
                fill_credit[0] -= cost

        def fill_flush():
            while fill_q:
                fill_q.popleft()[1]()
            fill_credit[0] = 0.0

        def proj_qk_m(w_tiles, bias_sb, dst_map, qu, use_bias, m, x_tiles):
            ps = ps_mm.tile([CH, QB], F32, tag="mm")
            for kt in range(NK):
                nc.tensor.matmul(
                    ps[:], w_tiles[kt][:, m * CH:(m + 1) * CH],
                    x_tiles[kt][:], start=(kt == 0),
                    stop=(not use_bias and kt == NK - 1))
            if use_bias:
                nc.tensor.matmul(
                    ps[:], bias_sb[0:1, m * CH:(m + 1) * CH],
                    ones_sb[0:1, :], start=False, stop=True)
            nc.vector.tensor_copy(out=dst_map[(m, qu)][:], in_=ps[:])

        def proj_qk_quarter(x_dram, w_tiles, bias_sb, dst_map, qu, use_bias,
                            push=False):
            x_tiles = load_x(x_dram, qu)
            for m in range(MT):
                if push:
                    fill_q.append((1800, lambda m=m: proj_qk_m(
                        w_tiles, bias_sb, dst_map, qu, use_bias, m, x_tiles)))
                else:
                    proj_qk_m(w_tiles, bias_sb, dst_map, qu, use_bias, m,
                              x_tiles)

        def proj_v_j(wv_tiles, qu, j, x_tiles):
            ps = ps_mm.tile([CH, DL], F32, tag="mm")
            for kt in range(NK):
                nc.tensor.matmul(
                    ps[:], x_tiles[kt][:, j * CH:(j + 1) * CH],
                    wv_tiles[kt][:], start=(kt == 0),
                    stop=(not has_bias[2] and kt == NK - 1))
            if has_bias[2]:
                nc.tensor.matmul(
                    ps[:], ones_sb[0:1, 0:CH], bv_sb[0:1, :],
                    start=False, stop=True)
            nc.vector.tensor_copy(
                out=v_g[qu][:, j, :, 0:DK],
                in_=ps[:].rearrange("p (h c) -> p h c", c=DK),
            )

        def proj_v_quarter(wv_tiles, qu, x_tiles=None, push=False):
            if x_tiles is None:
                x_tiles = load_x(xv, qu)
            # ones column for this quarter's V tiles
            nc.sync.dma_start(
                v_g[qu][:, :, :, DK:DK + 1],
                onesc.ap()[:, qu * NQU * HL:(qu + 1) * NQU * HL].rearrange(
                    "p (s h o) -> p s h o", h=HL, o=1),
            )
            for j in range(QB // CH):
                if push:
                    fill_q.append((1800, lambda j=j: proj_v_j(
                        wv_tiles, qu, j, x_tiles)))
                else:
                    proj_v_j(wv_tiles, qu, j, x_tiles)

        def attention_qb(qb):
            att = attp.tile([CH, MT, QB], BF16, tag="att")
            units = [(kt,) + plans[qb][kt] for kt in range(NKT)
                     if plans[qb][kt] is not None]
            units = [(kt, c0 * CH, nch * CH, mixed)
                     for (kt, c0, nch, mixed) in units]
            for j in range(NPAIR):
                raw_e = ps_raw.tile([CH, QB], F32, tag="raw")
                raw_o = ps_raw.tile([CH, QB], F32, tag="raw")
                nunit = 0
                for i0 in range(0, len(units), 2):
                    batch = units[i0:i0 + 2]
                    # scores for both packages first: the K=64 matmuls
                    # alternate PE row groups 0/64, so each weight load hides
                    # under the previous matmul and pairs run concurrently.
                    # the odd head packs at offset w when both windows fit in
                    # one PSUM bank (one exp call), else at the bank boundary
                    deficit = 0.0
                    scs = []
                    for (kt, o, w, mixed) in batch:
                        # the odd head always lands in its own PSUM bank:
                        # the two score matmuls run concurrently on disjoint
                        # row groups, and concurrent drains into ONE bank
                        # hard-fault the PSUM (observed as a device hang)
                        odd_off = QB
                        sc = ps_sc.tile([CH, 2 * QB], F32, tag="sc")
                        kt_tile = kt_q[(j, kt // 4)]
                        q_tile = qt_q[(j, qb)]
                        ktc = slice((kt % 4) * CH, (kt % 4 + 1) * CH)
                        for h01, base in ((0, 0), (1, odd_off)):
                            hp = h01 * DK
                            nc.tensor.matmul(
                                sc[:, base:base + w],
                                kt_tile[hp:hp + DK, ktc],
                                q_tile[hp:hp + DK, o:o + w],
                                start=True, stop=True)
                        scs.append((sc, odd_off))
                        ncalls = 1 if odd_off == w else 2
                        deficit += ((2 * w + 313 * ncalls) / 1.2
                                    - (1.2 * w / 2.4 + 2 * (w / 2.4 + 15)))
                    exs = []
                    for (kt, o, w, mixed), (sc, odd_off) in zip(batch, scs):
                        ex = expp.tile([CH, 2 * QB], BF16, tag="exp")
                        if odd_off == w:
                            nc.scalar.activation(ex[:, 0:2 * w], sc[:, 0:2 * w],
                                                 AF.Exp)
                        else:
                            nc.scalar.activation(ex[:, 0:w], sc[:, 0:w],
                                                 AF.Exp)
                            nc.scalar.activation(ex[:, QB:QB + w],
                                                 sc[:, QB:QB + w], AF.Exp)
                        if mixed:
                            exv = ex[:, 0:2 * odd_off].rearrange(
                                "p (a b) -> p a b", a=2)
                            for (rel, uidx) in mixed:
                                nc.vector.tensor_mul(
                                    exv[:, :, rel * CH:(rel + 1) * CH],
                                    exv[:, :, rel * CH:(rel + 1) * CH],
                                    mp_sb[:, uidx, :, :])
                        exs.append((ex, odd_off))
                    for (kt, o, w, mixed), (ex, odd_off) in zip(batch, exs):
                        for h01, base, raw in ((0, 0, raw_e),
                                               (1, odd_off, raw_o)):
                            nc.tensor.matmul(
                                raw[:, o:o + w],
                                v_g[kt // 4][:, kt % 4, 2 * j + h01, :],
                                ex[:, base:base + w],
                                start=(nunit == 0), stop=False,
                                skip_group_check=True)
                        nunit += 1
                    fill_step(deficit)
                # normalize pair -> attnT
                for h01, raw, asl in ((0, raw_e, slice(0, DK)),
                                      (1, raw_o, slice(DK, CH))):
                    # custom-DVE reciprocal needs an SBUF source: stage the
                    # denominator row out of PSUM first
                    den = recp.tile([1, QB], F32, tag="den")
                    if guard_empty_rows:
                        nc.vector.tensor_scalar_max(den[:], raw[DK:DK + 1, :],
                                                    1e-30)
                    else:
                        nc.vector.tensor_copy(den[:], raw[DK:DK + 1, :])
                    rec = recp.tile([1, QB], F32, tag="rec")
                    nc.vector.reciprocal_approx_fast(out=rec[:], in_=den[:])
                    recb = recp.tile([DK, QB], F32, tag="recb")
                    nc.gpsimd.partition_broadcast(recb[:], rec[:])
                    nc.vector.tensor_mul(att[asl, j, :], raw[0:DK, :], recb[:])
            return att

        def outproj_mo(qb, att, mo):
            ps = ps_mm.tile([CH, QB], F32, tag="mm")
            for ct in range(MT):
                nc.tensor.matmul(
                    ps[:], wo_all[:, ct, mo * CH:(mo + 1) * CH],
                    att[:, ct, :], start=(ct == 0), stop=(ct == MT - 1),
                    skip_group_check=True)
            ot = outp.tile([CH, QB], BF16, tag="ot")
            nc.vector.tensor_copy(out=ot[:], in_=ps[:])
            nc.sync.dma_start(
                outT.ap()[mo * CH:(mo + 1) * CH, qb * QB:(qb + 1) * QB],
                ot[:])

        def outproj_qb(qb, att, push=False):
            for mo in range(D // CH):
                if push:
                    fill_q.append((1000, lambda mo=mo: outproj_mo(qb, att, mo)))
                else:
                    outproj_mo(qb, att, mo)

        # ---- emission order sets scheduler priority: proj/out-proj matmuls
        # fill PE bubbles left by exp(ACT)-paced attention packages; the
        # out-projections are deferred so the attention tail still has PE
        # fill work.
        # interleave the first weight/x DMAs so the first projection matmul
        # can start as soon as one (w, x) tile pair lands
        wv_t = []
        xv0 = []
        for kt in range(NK):
            wtile = wt.tile([CH, DL], BF16, tag="w")
            nc.sync.dma_start(wtile[:], wv.ap()[kt * CH:(kt + 1) * CH, :])
            wv_t.append(wtile)
            xt = xin.tile([CH, QB], BF16, tag="x")
            nc.sync.dma_start(xt[:], xv.ap()[kt * CH:(kt + 1) * CH, 0:QB])
            xv0.append(xt)
        proj_v_quarter(wv_t, 0, x_tiles=xv0)
        wk_t = load_w(wk)
        proj_qk_quarter(xk, wk_t, bk_sb, kt_q, 0, has_bias[1])
        wq_t = load_w(wq)
        proj_qk_quarter(xq, wq_t, bq_sb, qt_q, 0, has_bias[0])
        # bulk constants needed from attention onward
        if n_patterns:
            nc.sync.dma_start(mp_sb[:], maskp.ap().rearrange(
                "p (u a f) -> p u a f", a=2, f=CH))
        nc.sync.dma_start(wo_all[:], wo.ap().rearrange("(t p) m -> p t m", p=CH))

        # direct emission; the scheduler fills attention's exp-paced PE
        # bubbles with the proj/out-proj matmuls emitted around it.  The
        # out-projections are deferred one attention block so the tail
        # still has PE fill work.
        proj_v_quarter(wv_t, 1)
        proj_qk_quarter(xk, wk_t, bk_sb, kt_q, 1, has_bias[1])
        proj_qk_quarter(xq, wq_t, bq_sb, qt_q, 1, has_bias[0])
        att0 = attention_qb(0)
        proj_v_quarter(wv_t, 2)
        proj_qk_quarter(xk, wk_t, bk_sb, kt_q, 2, has_bias[1])
        proj_qk_quarter(xq, wq_t, bq_sb, qt_q, 2, has_bias[0])
        att1 = attention_qb(1)
        proj_v_quarter(wv_t, 3)
        proj_qk_quarter(xk, wk_t, bk_sb, kt_q, 3, has_bias[1])
        proj_qk_quarter(xq, wq_t, bq_sb, qt_q, 3, has_bias[0])
        att2 = attention_qb(2)
        outproj_qb(0, att0)
        outproj_qb(1, att1)
        att3 = attention_qb(3)
        outproj_qb(2, att2)
        outproj_qb(3, att3)

    nc.compile()
    return nc


_CACHE = {}
LAST_RESULTS = None


def _install_ntff_shim():
    """Provide antenv.axon_hooks (NTFF profiling) when the image lacks it."""
    import sys, types, ctypes, contextlib
    if "antenv.axon_hooks" in sys.modules:
        return
    import antenv
    mod = types.ModuleType("antenv.axon_hooks")
    state = {"hook": None}
    mod.set_axon_ntff_profile_hook = lambda h: state.__setitem__("hook", h)
    mod.get_axon_ntff_profile_hook = lambda: state["hook"]
    sys.modules["antenv.axon_hooks"] = mod
    antenv.axon_hooks = mod
    try:
        lib = ctypes.CDLL("/opt/axon/libaxon_pjrt.so")
    except OSError:
        return
    if not hasattr(lib, "axon_start_nrt_profile"):
        return
    lib.axon_start_nrt_profile.argtypes = [
        ctypes.POINTER(ctypes.c_int64), ctypes.c_size_t]
    lib.axon_start_nrt_profile.restype = ctypes.c_int64
    lib.axon_stop_nrt_profile.argtypes = [ctypes.c_char_p]
    lib.axon_stop_nrt_profile.restype = ctypes.c_int64

    @contextlib.contextmanager
    def _hook(output_dir, device_ids):
        import jax
        jax.devices()
        if device_ids:
            ids = (ctypes.c_int64 * len(device_ids))(*device_ids)
            rc = lib.axon_start_nrt_profile(ids, len(device_ids))
        else:
            rc = lib.axon_start_nrt_profile(None, 0)
        if rc != 0:
            raise RuntimeError(f"axon_start_nrt_profile rc={rc}")
        try:
            yield
        finally:
            n = lib.axon_stop_nrt_profile(str(output_dir).encode())
            print(f"profile: {n} ntff file(s) in {output_dir}", file=sys.stderr)

    state["hook"] = _hook


def _get_nc(mask2d, has_bias):
    key = (hash(mask2d.tobytes()), has_bias)
    if key not in _CACHE:
        plans, patterns = _plan_from_mask(mask2d)
        # guard against fully-masked rows (reference maps softmax NaN -> 0)
        valid_any = (~mask2d).any(axis=1)
        guard = bool((~valid_any).any())
        _CACHE[key] = (_build(plans, len(patterns), guard, has_bias), patterns)
    return _CACHE[key]


def kernel(query, key, value, mask, Wq, bq, Wk, bk, Wv, bv, Wo, bo):
    from concourse.bass_utils import run_bass_kernel_spmd

    query = np.asarray(query, dtype=np.float32)
    key_ = np.asarray(key, dtype=np.float32)
    value = np.asarray(value, dtype=np.float32)
    mask2d = np.asarray(mask, dtype=bool).reshape(S, S)
    Wq = np.asarray(Wq, dtype=np.float32)
    Wk = np.asarray(Wk, dtype=np.float32)
    Wv = np.asarray(Wv, dtype=np.float32)
    Wo = np.asarray(Wo, dtype=np.float32)
    bq = np.asarray(bq, dtype=np.float32)
    bk = np.asarray(bk, dtype=np.float32)
    bv = np.asarray(bv, dtype=np.float32)
    bo = np.asarray(bo, dtype=np.float32)

    has_bias = (bool(bq.any()), bool(bk.any()), bool(bv.any()))
    nc, patterns = _get_nc(mask2d, has_bias)

    n_pat = len(patterns)
    if n_pat:
        mp = np.empty((CH, n_pat * 2 * CH), np.float32)
        for u, pat in enumerate(patterns):
            mp[:, (2 * u) * CH:(2 * u + 1) * CH] = pat
            mp[:, (2 * u + 1) * CH:(2 * u + 2) * CH] = pat
        mp = mp.astype(ml_dtypes.bfloat16)
    ones_row = np.ones((1, QB), ml_dtypes.bfloat16)
    ones_cols = np.ones((CH, NKT * HL), ml_dtypes.bfloat16)

    in_maps = []
    for c in range(NCORES):
        b, g = divmod(c, 2)
        gsl = slice(DL * g, DL * (g + 1))
        m = {
            "xq_t": _bf16(query[b].T),
            "xk_t": _bf16(key_[b].T),
            "xv_t": _bf16(value[b].T),
            "wq_t": _bf16(Wq[gsl].T * 0.125),
            "wk_t": _bf16(Wk[gsl].T),
            "wv_t": _bf16(Wv[gsl].T),
            "wo_t": _bf16(Wo[:, gsl].T),
            "bq8": _bf16(bq[gsl].reshape(1, DL) * 0.125),
            "bk": _bf16(bk[gsl].reshape(1, DL)),
            "bv": _bf16(bv[gsl].reshape(1, DL)),
            "ones_row": ones_row,
            "ones_cols": ones_cols,
        }
        if n_pat:
            m["maskp"] = mp
        in_maps.append(m)

    import os
    kwargs = {}
    if os.environ.get("BASS_MHA_TRACE"):
        _install_ntff_shim()
        tc_env = os.environ.get("BASS_MHA_TRACE_CORES", "0")
        cores = (list(range(NCORES)) if tc_env == "all"
                 else [int(x) for x in tc_env.split(",")])
        kwargs = dict(trace=True, trace_cores=cores)

    global LAST_RESULTS
    out = np.empty((B, S, D), np.float32)
    for attempt in range(3):
        res = run_bass_kernel_spmd(nc, in_maps, core_ids=list(range(NCORES)),
                                   **kwargs)
        LAST_RESULTS = res
        for b in range(B):
            acc = (res.results[2 * b]["outT"].astype(np.float32)
                   + res.results[2 * b + 1]["outT"].astype(np.float32))
            out[b] = acc.T
        # softmax-bounded outputs are O(10); a bad first execution after NEFF
        # load shows up as huge/NaN values -> rerun
        if np.isfinite(out).all() and np.abs(out).max() < 1e4:
            break
    return out + bo[None, None, :]


# revision 26
# speedup vs baseline: 1.0757x; 1.0439x over previous
"""Multi-head attention (B=4, S=2048, D=1024, H=16) on 8 TRN2 NeuronCores.

Sharding: core c handles batch b=c//2 and head-group g=c%2 (8 heads, 512 of
the 1024 model dims).  Wq/Wk/Wv column-parallel, Wo row-parallel; the two
head-group partial outputs per batch are summed on the host (no collectives).

Per-core dataflow (all matmuls bf16 in, fp32 PSUM accumulate):
  phase 1: Q.T = (Wq/8) @ x.T   [512,2048]
           K.T = Wk @ x.T       [512,2048]
           V   = x @ Wv.T       [2048,512]   stored head-major with a ones
                                             column and zero padding to 128:
                                             [128, 4, 8, 128] per quarter
  phase 2 (per 512-wide q-block, per HEAD PAIR (2j, 2j+1) sharing m-tile j):
           scoresT[k,q] for both heads of the pair land in one [128,1024]
           PSUM tile (even head at 0, odd at 512) via two K=64 matmuls on
           disjoint PE row groups (tile_position 0 / 64 -> concurrent);
           scores of two consecutive k-tiles are batched so the row groups
           alternate and the weight loads pipeline
           exp(ACT) over the packed tile, then one strided DVE multiply per
           diagonal chunk applies the 0/1 mask to both heads
           raw_h[128,512] += [V_h|1|0pad].T @ expT  (row 64 = softmax denom;
           FD=128 stationary keeps the weight load pipelined)
           attnT = raw[0:64] * (1/raw[64])  (fast reciprocal on the pair,
           gpsimd bcast, DVE mul)
  phase 3 (per q-block, deferred to fill the attention tail on PE):
           outT += Wo_g.T.T @ attnT_cat -> [1024,2048] bf16 partial
Host: out[b] = (partial_g0 + partial_g1).T + bo
"""

import numpy as np
import ml_dtypes
from collections import deque
from contextlib import ExitStack

B = 4
S = 2048
D = 1024
H = 16
DK = 64
G = 2                 # head groups
HL = H // G           # heads per core = 8
DL = D // G           # local head dims = 512
QB = 512              # q-block width
CH = 128              # chunk / k-tile width
NKT = S // CH         # 16 k-tiles
NQB = S // QB         # 4 q-blocks
NCORES = 8


def _bf16(x):
    return np.ascontiguousarray(x, dtype=np.float32).astype(ml_dtypes.bfloat16)


def _plan_from_mask(m):
    """m: [S, S] bool, True = masked (scores[q, k] masked).

    Returns (plans, patterns):
      plans[qb][kt] = None (skip) or (c0, nch, [(rel_chunk, uidx), ...])
        c0: first valid 128-chunk index within the q-block, nch: chunk count
      patterns: list of unique [128,128] float32 0/1 valid-masks (scoresT
        orientation: [k_partition, q_free]).
    """
    patterns = []
    pat_index = {}
    plans = []
    for qb in range(NQB):
        row = []
        for kt in range(NKT):
            # scoresT tile: partitions = k in [kt*128, ...), free = q chunk
            sub = m[qb * QB:(qb + 1) * QB, kt * CH:(kt + 1) * CH]  # [q, k]
            valid = (~sub).T  # [k, q] 128 x 512
            nchunks = QB // CH
            kinds = []
            for c in range(nchunks):
                ch = valid[:, c * CH:(c + 1) * CH]
                if ch.all():
                    kinds.append("full")
                elif not ch.any():
                    kinds.append("empty")
                else:
                    kinds.append("mixed")
            not_empty = [c for c in range(nchunks) if kinds[c] != "empty"]
            if not not_empty:
                row.append(None)
                continue
            c0, c1 = not_empty[0], not_empty[-1]
            mixed = []
            for c in range(c0, c1 + 1):
                if kinds[c] == "full":
                    continue
                pat = valid[:, c * CH:(c + 1) * CH].astype(np.float32)
                key = pat.tobytes()
                if key not in pat_index:
                    pat_index[key] = len(patterns)
                    patterns.append(pat)
                mixed.append((c - c0, pat_index[key]))
            row.append((c0, c1 - c0 + 1, mixed))
        plans.append(row)
    return plans, patterns


def _build(plans, n_patterns, guard_empty_rows, has_bias):
    import concourse.bacc as bacc
    import concourse.tile as tile
    from concourse import mybir

    F32 = mybir.dt.float32
    BF16 = mybir.dt.bfloat16
    AF = mybir.ActivationFunctionType

    nc = bacc.Bacc("TRN2", target_bir_lowering=False, debug=False)

    xq = nc.dram_tensor("xq_t", [D, S], BF16, kind="ExternalInput")
    xk = nc.dram_tensor("xk_t", [D, S], BF16, kind="ExternalInput")
    xv = nc.dram_tensor("xv_t", [D, S], BF16, kind="ExternalInput")
    wq = nc.dram_tensor("wq_t", [D, DL], BF16, kind="ExternalInput")
    wk = nc.dram_tensor("wk_t", [D, DL], BF16, kind="ExternalInput")
    wv = nc.dram_tensor("wv_t", [D, DL], BF16, kind="ExternalInput")
    wo = nc.dram_tensor("wo_t", [DL, D], BF16, kind="ExternalInput")
    bq = nc.dram_tensor("bq8", [1, DL], BF16, kind="ExternalInput")
    bk = nc.dram_tensor("bk", [1, DL], BF16, kind="ExternalInput")
    bv = nc.dram_tensor("bv", [1, DL], BF16, kind="ExternalInput")
    onesr = nc.dram_tensor("ones_row", [1, QB], BF16, kind="ExternalInput")
    onesc = nc.dram_tensor("ones_cols", [CH, NKT * HL], BF16, kind="ExternalInput")
    if n_patterns:
        # 0/1 valid patterns, duplicated x2 so one strided DVE multiply
        # masks both heads of a pair
        maskp = nc.dram_tensor("maskp", [CH, n_patterns * 2 * CH], BF16,
                               kind="ExternalInput")
    outT = nc.dram_tensor("outT", [D, S], BF16, kind="ExternalOutput")

    MT = DL // CH      # 4 dq/dcat tiles
    NQU = S // QB      # 4 s-quarters
    NK = D // CH       # 8 contraction tiles
    NPAIR = HL // 2    # 4 head pairs

    with tile.TileContext(nc) as tc, ExitStack() as ctx:
        persist = ctx.enter_context(tc.tile_pool(name="persist", bufs=1))
        xin = ctx.enter_context(tc.tile_pool(name="xin", bufs=24))
        wt = ctx.enter_context(tc.tile_pool(name="wt", bufs=25))
        expp = ctx.enter_context(tc.tile_pool(name="expp", bufs=4))
        attp = ctx.enter_context(tc.tile_pool(name="attp", bufs=4))
        outp = ctx.enter_context(tc.tile_pool(name="outp", bufs=4))
        recp = ctx.enter_context(tc.tile_pool(name="recp", bufs=4))
        ps_mm = ctx.enter_context(tc.tile_pool(name="ps_mm", bufs=2, space="PSUM"))
        ps_sc = ctx.enter_context(tc.tile_pool(name="ps_sc", bufs=2, space="PSUM"))
        ps_raw = ctx.enter_context(tc.tile_pool(name="ps_raw", bufs=2, space="PSUM"))

        # per-(m, quarter) projection output tiles -> fine-grained deps let
        # attention(qb) start as soon as quarters <= qb are projected
        qt_q = {(m, qu): persist.tile([CH, QB], BF16, name=f"qt_{m}_{qu}")
                for m in range(MT) for qu in range(NQU)}
        kt_q = {(m, qu): persist.tile([CH, QB], BF16, name=f"kt_{m}_{qu}")
                for m in range(MT) for qu in range(NQU)}
        # V stationary, head-major: [k, chunk j, head, V(64)|ones|zero pad]
        v_g = [persist.tile([CH, NQU, HL, CH], BF16, name=f"v_g{qu}")
               for qu in range(NQU)]
        wo_all = persist.tile([CH, MT, D], BF16)
        ones_sb = persist.tile([1, QB], BF16)
        bq_sb = persist.tile([1, DL], BF16)
        bk_sb = persist.tile([1, DL], BF16)
        bv_sb = persist.tile([1, DL], BF16)
        if n_patterns:
            mp_sb = persist.tile([CH, n_patterns, 2, CH], BF16)

        nc.sync.dma_start(ones_sb[:], onesr.ap())
        if has_bias[0]:
            nc.sync.dma_start(bq_sb[:], bq.ap())
        if has_bias[1]:
            nc.sync.dma_start(bk_sb[:], bk.ap())
        if has_bias[2]:
            nc.sync.dma_start(bv_sb[:], bv.ap())

        # PE warm-up while the first input DMAs land
        wu_ps = ps_mm.tile([1, QB], F32, tag="mm")
        for _ in range(12):
            nc.tensor.matmul(wu_ps[:], ones_sb[0:1, 0:1], ones_sb[0:1, :],
                             start=True, stop=True, skip_group_check=True)

        # zero the V stationary pad columns (gpsimd; idle during startup)
        for qu in range(NQU):
            nc.gpsimd.memset(v_g[qu][:, :, :, DK + 1:], 0.0)

        # weight tiles: loaded once, reused across quarters
        def load_w(dram):
            tiles = []
            for kt in range(NK):
                wtile = wt.tile([CH, DL], BF16, tag="w")
                nc.sync.dma_start(wtile[:], dram.ap()[kt * CH:(kt + 1) * CH, :])
                tiles.append(wtile)
            return tiles

        def load_x(dram, qu):
            tiles = []
            for kt in range(NK):
                xt = xin.tile([CH, QB], BF16, tag="x")
                nc.sync.dma_start(
                    xt[:], dram.ap()[kt * CH:(kt + 1) * CH,
                                     qu * QB:(qu + 1) * QB])
                tiles.append(xt)
            return tiles

        # ---- fill queue: projection / out-projection matmul bundles that
        # attention interleaves into its exp(ACT)-paced stretches so the PE
        # never idles and HAM stays warm.  Thunks emit instructions when
        # popped; emission order sets scheduler priority.
        fill_q = deque()
        fill_credit = [0.0]

        def fill_step(deficit_ns):
            fill_credit[0] += deficit_ns
            while fill_q and fill_credit[0] >= fill_q[0][0]:
                cost, thunk = fill_q.popleft()
                thunk()
                fill_credit[0] -= cost

        def fill_flush():
            while fill_q:
                fill_q.popleft()[1]()
            fill_credit[0] = 0.0

        def proj_qk_m(w_tiles, bias_sb, dst_map, qu, use_bias, m, x_tiles):
            ps = ps_mm.tile([CH, QB], F32, tag="mm")
            for kt in range(NK):
                nc.tensor.matmul(
                    ps[:], w_tiles[kt][:, m * CH:(m + 1) * CH],
                    x_tiles[kt][:], start=(kt == 0),
                    stop=(not use_bias and kt == NK - 1))
            if use_bias:
                nc.tensor.matmul(
                    ps[:], bias_sb[0:1, m * CH:(m + 1) * CH],
                    ones_sb[0:1, :], start=False, stop=True)
            nc.vector.tensor_copy(out=dst_map[(m, qu)][:], in_=ps[:])

        def proj_qk_quarter(x_dram, w_tiles, bias_sb, dst_map, qu, use_bias,
                            push=False):
            x_tiles = load_x(x_dram, qu)
            for m in range(MT):
                if push:
                    fill_q.append((1800, lambda m=m: proj_qk_m(
                        w_tiles, bias_sb, dst_map, qu, use_bias, m, x_tiles)))
                else:
                    proj_qk_m(w_tiles, bias_sb, dst_map, qu, use_bias, m,
                              x_tiles)

        def proj_v_j(wv_tiles, qu, j, x_tiles):
            ps = ps_mm.tile([CH, DL], F32, tag="mm")
            for kt in range(NK):
                nc.tensor.matmul(
                    ps[:], x_tiles[kt][:, j * CH:(j + 1) * CH],
                    wv_tiles[kt][:], start=(kt == 0),
                    stop=(not has_bias[2] and kt == NK - 1))
            if has_bias[2]:
                nc.tensor.matmul(
                    ps[:], ones_sb[0:1, 0:CH], bv_sb[0:1, :],
                    start=False, stop=True)
            nc.vector.tensor_copy(
                out=v_g[qu][:, j, :, 0:DK],
                in_=ps[:].rearrange("p (h c) -> p h c", c=DK),
            )

        def proj_v_quarter(wv_tiles, qu, x_tiles=None, push=False):
            if x_tiles is None:
                x_tiles = load_x(xv, qu)
            # ones column for this quarter's V tiles
            nc.sync.dma_start(
                v_g[qu][:, :, :, DK:DK + 1],
                onesc.ap()[:, qu * NQU * HL:(qu + 1) * NQU * HL].rearrange(
                    "p (s h o) -> p s h o", h=HL, o=1),
            )
            for j in range(QB // CH):
                if push:
                    fill_q.append((1800, lambda j=j: proj_v_j(
                        wv_tiles, qu, j, x_tiles)))
                else:
                    proj_v_j(wv_tiles, qu, j, x_tiles)

        def attention_qb(qb):
            att = attp.tile([CH, MT, QB], BF16, tag="att")
            units = [(kt,) + plans[qb][kt] for kt in range(NKT)
                     if plans[qb][kt] is not None]
            units = [(kt, c0 * CH, nch * CH, mixed)
                     for (kt, c0, nch, mixed) in units]
            for j in range(NPAIR):
                raw_e = ps_raw.tile([CH, QB], F32, tag="raw")
                raw_o = ps_raw.tile([CH, QB], F32, tag="raw")
                nunit = 0
                for i0 in range(0, len(units), 2):
                    batch = units[i0:i0 + 2]
                    # scores for both packages first: the K=64 matmuls
                    # alternate PE row groups 0/64, so each weight load hides
                    # under the previous matmul and pairs run concurrently.
                    # the odd head packs at offset w when both windows fit in
                    # one PSUM bank (one exp call), else at the bank boundary
                    deficit = 0.0
                    scs = []
                    for (kt, o, w, mixed) in batch:
                        # the odd head always lands in its own PSUM bank:
                        # the two score matmuls run concurrently on disjoint
                        # row groups, and concurrent drains into ONE bank
                        # hard-fault the PSUM (observed as a device hang)
                        odd_off = QB
                        sc = ps_sc.tile([CH, 2 * QB], F32, tag="sc")
                        kt_tile = kt_q[(j, kt // 4)]
                        q_tile = qt_q[(j, qb)]
                        ktc = slice((kt % 4) * CH, (kt % 4 + 1) * CH)
                        for h01, base in ((0, 0), (1, odd_off)):
                            hp = h01 * DK
                            nc.tensor.matmul(
                                sc[:, base:base + w],
                                kt_tile[hp:hp + DK, ktc],
                                q_tile[hp:hp + DK, o:o + w],
                                start=True, stop=True)
                        scs.append((sc, odd_off))
                        ncalls = 1 if odd_off == w else 2
                        deficit += ((2 * w + 313 * ncalls) / 1.2
                                    - (1.2 * w / 2.4 + 2 * (w / 2.4 + 15)))
                    exs = []
                    for (kt, o, w, mixed), (sc, odd_off) in zip(batch, scs):
                        ex = expp.tile([CH, 2 * QB], BF16, tag="exp")
                        if odd_off == w:
                            nc.scalar.activation(ex[:, 0:2 * w], sc[:, 0:2 * w],
                                                 AF.Exp)
                        else:
                            nc.scalar.activation(ex[:, 0:w], sc[:, 0:w],
                                                 AF.Exp)
                            nc.scalar.activation(ex[:, QB:QB + w],
                                                 sc[:, QB:QB + w], AF.Exp)
                        if mixed:
                            exv = ex[:, 0:2 * odd_off].rearrange(
                                "p (a b) -> p a b", a=2)
                            for (rel, uidx) in mixed:
                                nc.vector.tensor_mul(
                                    exv[:, :, rel * CH:(rel + 1) * CH],
                                    exv[:, :, rel * CH:(rel + 1) * CH],
                                    mp_sb[:, uidx, :, :])
                        exs.append((ex, odd_off))
                    for (kt, o, w, mixed), (ex, odd_off) in zip(batch, exs):
                        for h01, base, raw in ((0, 0, raw_e),
                                               (1, odd_off, raw_o)):
                            nc.tensor.matmul(
                                raw[:, o:o + w],
                                v_g[kt // 4][:, kt % 4, 2 * j + h01, :],
                                ex[:, base:base + w],
                                start=(nunit == 0), stop=False,
                                skip_group_check=True)
                        nunit += 1
                    fill_step(deficit)
                # normalize pair -> attnT
                for h01, raw, asl in ((0, raw_e, slice(0, DK)),
                                      (1, raw_o, slice(DK, CH))):
                    # custom-DVE reciprocal needs an SBUF source: stage the
                    # denominator row out of PSUM first
                    den = recp.tile([1, QB], F32, tag="den")
                    if guard_empty_rows:
                        nc.vector.tensor_scalar_max(den[:], raw[DK:DK + 1, :],
                                                    1e-30)
                    else:
                        nc.vector.tensor_copy(den[:], raw[DK:DK + 1, :])
                    rec = recp.tile([1, QB], F32, tag="rec")
                    nc.vector.reciprocal_approx_fast(out=rec[:], in_=den[:])
                    recb = recp.tile([DK, QB], F32, tag="recb")
                    nc.gpsimd.partition_broadcast(recb[:], rec[:])
                    nc.vector.tensor_mul(att[asl, j, :], raw[0:DK, :], recb[:])
            return att

        def outproj_mo(qb, att, mo):
            ps = ps_mm.tile([CH, QB], F32, tag="mm")
            for ct in range(MT):
                nc.tensor.matmul(
                    ps[:], wo_all[:, ct, mo * CH:(mo + 1) * CH],
                    att[:, ct, :], start=(ct == 0), stop=(ct == MT - 1),
                    skip_group_check=True)
            ot = outp.tile([CH, QB], BF16, tag="ot")
            nc.vector.tensor_copy(out=ot[:], in_=ps[:])
            nc.sync.dma_start(
                outT.ap()[mo * CH:(mo + 1) * CH, qb * QB:(qb + 1) * QB],
                ot[:])

        def outproj_qb(qb, att, push=False):
            for mo in range(D // CH):
                if push:
                    fill_q.append((1000, lambda mo=mo: outproj_mo(qb, att, mo)))
                else:
                    outproj_mo(qb, att, mo)

        # ---- emission order sets scheduler priority: proj/out-proj matmuls
        # fill PE bubbles left by exp(ACT)-paced attention packages; the
        # out-projections are deferred so the attention tail still has PE
        # fill work.
        # interleave the first weight/x DMAs so the first projection matmul
        # can start as soon as one (w, x) tile pair lands
        wv_t = []
        xv0 = []
        for kt in range(NK):
            wtile = wt.tile([CH, DL], BF16, tag="w")
            nc.sync.dma_start(wtile[:], wv.ap()[kt * CH:(kt + 1) * CH, :])
            wv_t.append(wtile)
            xt = xin.tile([CH, QB], BF16, tag="x")
            nc.sync.dma_start(xt[:], xv.ap()[kt * CH:(kt + 1) * CH, 0:QB])
            xv0.append(xt)
        proj_v_quarter(wv_t, 0, x_tiles=xv0)
        wk_t = load_w(wk)
        proj_qk_quarter(xk, wk_t, bk_sb, kt_q, 0, has_bias[1])
        wq_t = load_w(wq)
        proj_qk_quarter(xq, wq_t, bq_sb, qt_q, 0, has_bias[0])
        # bulk constants needed from attention onward
        if n_patterns:
            nc.sync.dma_start(mp_sb[:], maskp.ap().rearrange(
                "p (u a f) -> p u a f", a=2, f=CH))
        nc.sync.dma_start(wo_all[:], wo.ap().rearrange("(t p) m -> p t m", p=CH))

        # later projection rounds and out-projections go through the fill
        # queue: attention pops ~one matmul bundle per exp-paced batch, so
        # the fill lands exactly in the PE bubbles at the right scheduler
        # priority (a whole block emitted after an attention block either
        # runs too early or starves the exp stream).  x DMAs are issued
        # lazily at pop time.
        def lazy_x(dram, qu, extra=None):
            cell = {}

            def get():
                if "t" not in cell:
                    cell["t"] = load_x(dram, qu)
                    if extra is not None:
                        extra()
                return cell["t"]
            return get

        def push_round(qu):
            def ones_dma():
                nc.sync.dma_start(
                    v_g[qu][:, :, :, DK:DK + 1],
                    onesc.ap()[:, qu * NQU * HL:(qu + 1) * NQU * HL].rearrange(
                        "p (s h o) -> p s h o", h=HL, o=1))
            gxv = lazy_x(xv, qu, ones_dma)
            gxk = lazy_x(xk, qu)
            gxq = lazy_x(xq, qu)
            for j in range(QB // CH):
                fill_q.append((1800, lambda j=j: proj_v_j(wv_t, qu, j, gxv())))
            for m in range(MT):
                fill_q.append((1800, lambda m=m: proj_qk_m(
                    wk_t, bk_sb, kt_q, qu, has_bias[1], m, gxk())))
            for m in range(MT):
                fill_q.append((1800, lambda m=m: proj_qk_m(
                    wq_t, bq_sb, qt_q, qu, has_bias[0], m, gxq())))

        def push_outproj(qb, att):
            for mo in range(D // CH):
                fill_q.append((1000, lambda mo=mo: outproj_mo(qb, att, mo)))

        proj_v_quarter(wv_t, 1)
        proj_qk_quarter(xk, wk_t, bk_sb, kt_q, 1, has_bias[1])
        proj_qk_quarter(xq, wq_t, bq_sb, qt_q, 1, has_bias[0])
        push_round(2)
        att0 = attention_qb(0)
        att1 = attention_qb(1)
        fill_flush()            # round 2 complete before attention(2)
        push_round(3)
        push_outproj(0, att0)
        att2 = attention_qb(2)
        fill_flush()            # round 3 complete before attention(3)
        push_outproj(1, att1)
        push_outproj(2, att2)
        att3 = attention_qb(3)
        fill_flush()
        outproj_qb(3, att3)

    nc.compile()
    return nc


_CACHE = {}
LAST_RESULTS = None


def _install_ntff_shim():
    """Provide antenv.axon_hooks (NTFF profiling) when the image lacks it."""
    import sys, types, ctypes, contextlib
    if "antenv.axon_hooks" in sys.modules:
        return
    import antenv
    mod = types.ModuleType("antenv.axon_hooks")
    state = {"hook": None}
    mod.set_axon_ntff_profile_hook = lambda h: state.__setitem__("hook", h)
    mod.get_axon_ntff_profile_hook = lambda: state["hook"]
    sys.modules["antenv.axon_hooks"] = mod
    antenv.axon_hooks = mod
    try:
        lib = ctypes.CDLL("/opt/axon/libaxon_pjrt.so")
    except OSError:
        return
    if not hasattr(lib, "axon_start_nrt_profile"):
        return
    lib.axon_start_nrt_profile.argtypes = [
        ctypes.POINTER(ctypes.c_int64), ctypes.c_size_t]
    lib.axon_start_nrt_profile.restype = ctypes.c_int64
    lib.axon_stop_nrt_profile.argtypes = [ctypes.c_char_p]
    lib.axon_stop_nrt_profile.restype = ctypes.c_int64

    @contextlib.contextmanager
    def _hook(output_dir, device_ids):
        import jax
        jax.devices()
        if device_ids:
            ids = (ctypes.c_int64 * len(device_ids))(*device_ids)
            rc = lib.axon_start_nrt_profile(ids, len(device_ids))
        else:
            rc = lib.axon_start_nrt_profile(None, 0)
        if rc != 0:
            raise RuntimeError(f"axon_start_nrt_profile rc={rc}")
        try:
            yield
        finally:
            n = lib.axon_stop_nrt_profile(str(output_dir).encode())
            print(f"profile: {n} ntff file(s) in {output_dir}", file=sys.stderr)

    state["hook"] = _hook


def _get_nc(mask2d, has_bias):
    key = (hash(mask2d.tobytes()), has_bias)
    if key not in _CACHE:
        plans, patterns = _plan_from_mask(mask2d)
        # guard against fully-masked rows (reference maps softmax NaN -> 0)
        valid_any = (~mask2d).any(axis=1)
        guard = bool((~valid_any).any())
        _CACHE[key] = (_build(plans, len(patterns), guard, has_bias), patterns)
    return _CACHE[key]


def kernel(query, key, value, mask, Wq, bq, Wk, bk, Wv, bv, Wo, bo):
    from concourse.bass_utils import run_bass_kernel_spmd

    query = np.asarray(query, dtype=np.float32)
    key_ = np.asarray(key, dtype=np.float32)
    value = np.asarray(value, dtype=np.float32)
    mask2d = np.asarray(mask, dtype=bool).reshape(S, S)
    Wq = np.asarray(Wq, dtype=np.float32)
    Wk = np.asarray(Wk, dtype=np.float32)
    Wv = np.asarray(Wv, dtype=np.float32)
    Wo = np.asarray(Wo, dtype=np.float32)
    bq = np.asarray(bq, dtype=np.float32)
    bk = np.asarray(bk, dtype=np.float32)
    bv = np.asarray(bv, dtype=np.float32)
    bo = np.asarray(bo, dtype=np.float32)

    has_bias = (bool(bq.any()), bool(bk.any()), bool(bv.any()))
    nc, patterns = _get_nc(mask2d, has_bias)

    n_pat = len(patterns)
    if n_pat:
        mp = np.empty((CH, n_pat * 2 * CH), np.float32)
        for u, pat in enumerate(patterns):
            mp[:, (2 * u) * CH:(2 * u + 1) * CH] = pat
            mp[:, (2 * u + 1) * CH:(2 * u + 2) * CH] = pat
        mp = mp.astype(ml_dtypes.bfloat16)
    ones_row = np.ones((1, QB), ml_dtypes.bfloat16)
    ones_cols = np.ones((CH, NKT * HL), ml_dtypes.bfloat16)

    in_maps = []
    for c in range(NCORES):
        b, g = divmod(c, 2)
        gsl = slice(DL * g, DL * (g + 1))
        m = {
            "xq_t": _bf16(query[b].T),
            "xk_t": _bf16(key_[b].T),
            "xv_t": _bf16(value[b].T),
            "wq_t": _bf16(Wq[gsl].T * 0.125),
            "wk_t": _bf16(Wk[gsl].T),
            "wv_t": _bf16(Wv[gsl].T),
            "wo_t": _bf16(Wo[:, gsl].T),
            "bq8": _bf16(bq[gsl].reshape(1, DL) * 0.125),
            "bk": _bf16(bk[gsl].reshape(1, DL)),
            "bv": _bf16(bv[gsl].reshape(1, DL)),
            "ones_row": ones_row,
            "ones_cols": ones_cols,
        }
        if n_pat:
            m["maskp"] = mp
        in_maps.append(m)

    import os
    kwargs = {}
    if os.environ.get("BASS_MHA_TRACE"):
        _install_ntff_shim()
        tc_env = os.environ.get("BASS_MHA_TRACE_CORES", "0")
        cores = (list(range(NCORES)) if tc_env == "all"
                 else [int(x) for x in tc_env.split(",")])
        kwargs = dict(trace=True, trace_cores=cores)

    global LAST_RESULTS
    out = np.empty((B, S, D), np.float32)
    for attempt in range(3):
        res = run_bass_kernel_spmd(nc, in_maps, core_ids=list(range(NCORES)),
                                   **kwargs)
        LAST_RESULTS = res
        for b in range(B):
            acc = (res.results[2 * b]["outT"].astype(np.float32)
                   + res.results[2 * b + 1]["outT"].astype(np.float32))
            out[b] = acc.T
        # softmax-bounded outputs are O(10); a bad first execution after NEFF
        # load shows up as huge/NaN values -> rerun
        if np.isfinite(out).all() and np.abs(out).max() < 1e4:
            break
    return out + bo[None, None, :]
